# revision 1
# baseline (speedup 1.0000x reference)
"""GCN (3-layer + MLP head) on 8 Trainium2 NeuronCores.

Strategy (graph-parallel, per sharding hint):
  - Nodes sharded 8 ways by id; each core owns its dst-node shard plus the
    edges (incl. added self-loops) pointing into it.
  - Phase A (per core): Hs = dinv * (X_shard @ W1), written as a bf16 row
    table; AllGather -> full node table in every core's HBM.
  - MP phase: edges sorted by (src-quarter, dst); dma_gather pulls 256B bf16
    rows; per 128-edge group a one-hot S matrix (built with a single DVE
    tensor_scalar is_equal against an iota row) scatter-sums messages into
    the dst-block via a PE matmul accumulated in PSUM.
  - GCN layers 2+3 only feed a global mean, so they collapse to weighted
    node sums with host-precomputed normalization vectors (u2, S1, S2);
    p = sum_d u2[d]*relu(h1[d]) is a PE reduction, AllReduced across cores.
  - The tiny MLP head runs replicated on-device; core 0's output is returned.
"""
import math
import numpy as np
import ml_dtypes

import concourse.bass as bass
import concourse.tile as tile
from concourse import bacc, mybir
from concourse.bass_utils import run_bass_kernel_spmd

N_CORES = 8
F = 128          # feature dim (all layers)
BLK = 128        # dst-block size (PSUM partition dim)
GRP = 128        # edges per matmul group (PE contraction dim)
CHUNK_GROUPS = 16            # groups per gather chunk
CHUNK = CHUNK_GROUPS * GRP   # idxs per dma_gather call (2048)
NQUEUES = 4
PAD_DST = 130.0  # dst offset sentinel for padded edge slots (!= 0..127)

BF16 = ml_dtypes.bfloat16


# ----------------------------------------------------------------------------
# host preprocessing: shard, normalize, sort, pad, schedule
# ----------------------------------------------------------------------------
def _preprocess(graph, edge_index, rates, params):
    N = graph.shape[0]
    assert N % N_CORES == 0
    shard = N // N_CORES                      # real nodes per core
    nb = math.ceil(shard / BLK)               # dst blocks per core
    shard_pad = nb * BLK
    nq = 4                                    # src quarters (int16 idx range)
    assert N_CORES % nq == 0
    qshards = N_CORES // nq                   # shards per quarter
    qrows = qshards * shard_pad               # padded table rows per quarter
    assert qrows - 1 <= np.iinfo(np.int16).max

    src = np.asarray(edge_index[0], np.int64)
    dst = np.asarray(edge_index[1], np.int64)
    E = src.shape[0]

    # degrees / normalization (float64 host precompute of scalar edge data)
    deg = np.bincount(dst, minlength=N).astype(np.float64) + 1.0
    dinv = deg ** -0.5
    sq = deg ** 0.5
    u1 = dinv * (np.bincount(src, weights=dinv[dst], minlength=N) + dinv)
    y = u1 * dinv
    u2 = dinv * (np.bincount(src, weights=y[dst], minlength=N) + y)
    S1 = float(u1.sum())
    S2 = float(u2.sum())

    # add self edges
    allnodes = np.arange(N, dtype=np.int64)
    src_a = np.concatenate([src, allnodes])
    dst_a = np.concatenate([dst, allnodes])

    # per-core edge lists (by dst shard)
    core_of = dst_a // shard
    table_row = (src_a // shard) * shard_pad + (src_a % shard)
    quarter = table_row // qrows
    loc_idx = (table_row % qrows).astype(np.int64)
    dl = dst_a % shard
    blk_of = dl // BLK
    off_of = (dl % BLK).astype(np.int64)

    # counts per (core, quarter, block)
    cnt = np.zeros((N_CORES, nq, nb), np.int64)
    np.add.at(cnt, (core_of, quarter, blk_of), 1)
    cnt_max = cnt.max(axis=0)                       # [nq, nb]
    assert (cnt.min(axis=0) > 0).all(), "empty (quarter, block) cell"
    g_cell = -(-cnt_max // GRP)                     # groups per cell [nq, nb]

    # chunk-level schedule, identical for every core
    # stream: quarter 0 blocks 0..nb-1, quarter 1 ..., each cell g_cell groups
    sched_chunks = []   # list of chunks; chunk = (quarter, [ (block|-1, flags) x64 ])
    first_seen = set()
    entries_all = []
    for q in range(nq):
        entries = []
        for b in range(nb):
            g = int(g_cell[q, b])
            for k in range(g):
                flags = dict(
                    bias=(b not in first_seen),
                    qstart=(k == 0),
                    qend=(k == g - 1),
                    final=(q == nq - 1 and k == g - 1),
                )
                first_seen.add(b)
                entries.append((b, flags))
                flags["bias"] = flags["bias"] and k == 0
        # pad to chunk multiple with dummy groups
        while len(entries) % CHUNK_GROUPS:
            entries.append((-1, None))
        for i in range(0, len(entries), CHUNK_GROUPS):
            sched_chunks.append((q, entries[i : i + CHUNK_GROUPS]))
        entries_all.append(entries)
    nchunk = len(sched_chunks)

    # per-core edge data arrays in schedule order
    idx16 = np.zeros((N_CORES, nchunk, 128, CHUNK // 16), np.int16)
    dstid = np.full((N_CORES, nchunk, 128, CHUNK_GROUPS), PAD_DST, np.float32)
    for c in range(N_CORES):
        m = core_of == c
        q_c, b_c = quarter[m], blk_of[m]
        il_c, off_c = loc_idx[m], off_of[m]
        # sort by (quarter, block, arbitrary)
        order = np.lexsort((b_c, q_c))
        q_c, b_c, il_c, off_c = q_c[order], b_c[order], il_c[order], off_c[order]
        # cell boundaries
        ptr = 0
        # precompute per-cell slices
        cell_start = {}
        for i in range(len(q_c)):
            key = (int(q_c[i]), int(b_c[i]))
            if key not in cell_start:
                cell_start[key] = i
        cell_cnt = {(int(qq), int(bb)): int(cnt[c, qq, bb])
                    for qq in range(nq) for bb in range(nb)}
        # walk the schedule, fill slots
        iv = np.zeros(CHUNK, np.int64)
        ov = np.zeros(CHUNK, np.float64)
        pos_in_cell = {}
        for ch, (q, entries) in enumerate(sched_chunks):
            iv[:] = 0
            ov[:] = PAD_DST
            for g, (b, flags) in enumerate(entries):
                if b < 0:
                    continue
                s0 = cell_start.get((q, b))
                used = pos_in_cell.get((q, b), 0)
                ncell = cell_cnt[(q, b)]
                take = min(GRP, ncell - used)
                if take > 0:
                    sl = slice(s0 + used, s0 + used + take)
                    iv[g * GRP : g * GRP + take] = il_c[sl]
                    ov[g * GRP : g * GRP + take] = off_c[sl]
                    pos_in_cell[(q, b)] = used + take
            e = np.arange(CHUNK)
            base = idx16[c, ch]
            tmp = np.zeros((16, CHUNK // 16), np.int16)
            tmp[e % 16, e // 16] = iv
            base[:] = np.tile(tmp, (8, 1))
            dstid[c, ch][e % 128, e // 128] = ov.astype(np.float32)
        for q in range(nq):
            for b in range(nb):
                assert pos_in_cell.get((q, b), 0) == cell_cnt[(q, b)]

    # phase A inputs
    X = np.asarray(graph, np.float32)
    xt = np.zeros((N_CORES, F, shard_pad), np.float32)
    dinv_pm = np.zeros((N_CORES, BLK, nb), np.float32)
    u2_pm = np.zeros((N_CORES, BLK, nb), np.float32)
    sqdeg = np.zeros((N_CORES, 1, shard_pad), np.float32)
    for c in range(N_CORES):
        xs = X[c * shard : (c + 1) * shard]
        xt[c, :, :shard] = xs.T
        dv = np.zeros(shard_pad); dv[:shard] = dinv[c * shard : (c + 1) * shard]
        uv = np.zeros(shard_pad); uv[:shard] = u2[c * shard : (c + 1) * shard]
        sv = np.zeros(shard_pad); sv[:shard] = sq[c * shard : (c + 1) * shard]
        dinv_pm[c] = dv.reshape(nb, BLK).T
        u2_pm[c] = uv.reshape(nb, BLK).T
        sqdeg[c, 0] = sv

    p = params
    col = lambda v: np.asarray(v, np.float32).reshape(-1, 1)
    iota = np.tile(np.arange(BLK, dtype=np.float32)[None, :], (128, 1))
    common = dict(
        w1=np.asarray(p["conv1_W"], np.float32),
        b1row=np.asarray(p["conv1_b"], np.float32).reshape(1, F),
        iota=iota,
        rates_col=col(rates),
        encw1=np.asarray(p["enc_W1"], np.float32),
        encb1=col(p["enc_b1"]),
        encw2=np.asarray(p["enc_W2"], np.float32),
        encb2=col(p["enc_b2"]),
        w2a=np.asarray(p["conv2_W"], np.float32)[:F],
        w2b=np.asarray(p["conv2_W"], np.float32)[F:],
        b2col=col(p["conv2_b"]),
        s1col=np.full((F, 1), S1, np.float32),
        s2col=np.full((F, 1), S2, np.float32),
        w3=np.asarray(p["conv3_W"], np.float32),
        b3col=col(p["conv3_b"]),
        hidw=np.asarray(p["hid_W"], np.float32),
        hidb=np.asarray(p["hid_b"], np.float32).reshape(2, F).T,
        hid2wa=np.asarray(p["hid2_W"], np.float32)[:F],
        hid2wb=np.asarray(p["hid2_W"], np.float32)[F:],
        hid2b=col(p["hid2_b"]),
        finw=np.asarray(p["fin_W"], np.float32),
        finb=col(p["fin_b"]),
    )
    in_maps = []
    for c in range(N_CORES):
        m = dict(common)
        m.update(
            xt=xt[c], sqdeg=sqdeg[c], dinv=dinv_pm[c], u2c=u2_pm[c],
            idx16=idx16[c], dstid=dstid[c],
        )
        in_maps.append(m)

    dims = dict(N=N, shard=shard, shard_pad=shard_pad, nb=nb, nq=nq,
                qrows=qrows, nchunk=nchunk)
    return in_maps, sched_chunks, dims


# ----------------------------------------------------------------------------
# device program
# ----------------------------------------------------------------------------
def _build(sched_chunks, dims):
    nb, nq, qrows, nchunk = dims["nb"], dims["nq"], dims["qrows"], dims["nchunk"]
    shard_pad, N = dims["shard_pad"], dims["N"]
    rows_total = N_CORES * shard_pad
    f32, bf16, i16 = mybir.dt.float32, mybir.dt.bfloat16, mybir.dt.int16

    nc = bacc.Bacc("TRN2", target_bir_lowering=False, debug=False,
                   num_devices=N_CORES, num_swdge_queues=NQUEUES)
    I = lambda name, shape, dt=f32: nc.dram_tensor(name, shape, dt, kind="ExternalInput")
    xt_e = I("xt", [F, shard_pad])
    w1_e = I("w1", [F, F]); b1_e = I("b1row", [1, F])
    sq_e = I("sqdeg", [1, shard_pad])
    dinv_e = I("dinv", [BLK, nb]); u2_e = I("u2c", [BLK, nb])
    idx_e = I("idx16", [nchunk, 128, CHUNK // 16], i16)
    dst_e = I("dstid", [nchunk, 128, CHUNK_GROUPS])
    iota_e = I("iota", [128, BLK])
    rates_e = I("rates_col", [16, 1])
    encw1_e = I("encw1", [16, 8]); encb1_e = I("encb1", [8, 1])
    encw2_e = I("encw2", [8, F]); encb2_e = I("encb2", [F, 1])
    w2a_e = I("w2a", [F, F]); w2b_e = I("w2b", [F, F]); b2_e = I("b2col", [F, 1])
    s1_e = I("s1col", [F, 1]); s2_e = I("s2col", [F, 1])
    w3_e = I("w3", [F, F]); b3_e = I("b3col", [F, 1])
    hidw_e = I("hidw", [F, 2 * F]); hidb_e = I("hidb", [F, 2])
    hid2wa_e = I("hid2wa", [F, F]); hid2wb_e = I("hid2wb", [F, F])
    hid2b_e = I("hid2b", [F, 1])
    finw_e = I("finw", [F, 2]); finb_e = I("finb", [2, 1])
    out_e = nc.dram_tensor("out", [2, 1], f32, kind="ExternalOutput")
    import os
    DBG = bool(int(os.environ.get("GCN_DEBUG", "0")))
    if DBG:
        hs_dbg = nc.dram_tensor("hs_dbg", [N_CORES * dims["shard_pad"], F], bf16, kind="ExternalOutput")
        acc_dbg = nc.dram_tensor("acc_dbg", [BLK, dims["nb"], F], f32, kind="ExternalOutput")
        p_dbg = nc.dram_tensor("p_dbg", [F, 1], f32, kind="ExternalOutput")
        ppre_dbg = nc.dram_tensor("ppre_dbg", [1, F], f32, kind="ExternalOutput")

    hs_shard = nc.dram_tensor("hs_shard", [shard_pad, F], bf16)
    hs_full = nc.dram_tensor("hs_full", [rows_total, F], bf16, addr_space="Shared")
    p_dram = nc.dram_tensor("p_dram", [1, F], f32)
    p_shared = nc.dram_tensor("p_shared", [1, F], f32, addr_space="Shared")
    groups_all = list(range(N_CORES))

    with tile.TileContext(nc) as tc:
        with (
            tc.tile_pool(name="const", bufs=1) as cpool,
            tc.tile_pool(name="xa", bufs=3) as xapool,
            tc.tile_pool(name="work", bufs=8) as wpool,
            tc.tile_pool(name="gat", bufs=8) as gpool,
            tc.tile_pool(name="sstile", bufs=4) as spool,
            tc.tile_pool(name="accum", bufs=1) as apool,
            tc.tile_pool(name="ps", bufs=3, space="PSUM") as pspool,
            tc.tile_pool(name="psp", bufs=1, space="PSUM") as psppool,
        ):
            # ---- constants / small inputs
            w1_sb = cpool.tile([F, F], f32); nc.sync.dma_start(w1_sb[:], w1_e[:])
            b1_sb = cpool.tile([1, F], f32); nc.sync.dma_start(b1_sb[:], b1_e[:])
            sq_sb = cpool.tile([1, shard_pad], f32); nc.sync.dma_start(sq_sb[:], sq_e[:])
            dinv_sb = cpool.tile([BLK, nb], f32); nc.sync.dma_start(dinv_sb[:], dinv_e[:])
            u2_sb = cpool.tile([BLK, nb], f32); nc.sync.dma_start(u2_sb[:], u2_e[:])
            iota_sb = cpool.tile([128, BLK], f32); nc.sync.dma_start(iota_sb[:], iota_e[:])

            # ---- phase A: Hs = dinv * (X @ W1) -> bf16 table shard
            for n in range(nb):
                xts = xapool.tile([F, BLK], f32, tag="xts")
                nc.sync.dma_start(xts[:], xt_e[:, n * BLK : (n + 1) * BLK])
                psA = pspool.tile([BLK, F], f32, tag="ps")
                nc.tensor.matmul(psA[:], xts[:], w1_sb[:], start=True, stop=True)
                hsa = xapool.tile([BLK, F], bf16, tag="hsa")
                nc.scalar.activation(
                    hsa[:], psA[:], mybir.ActivationFunctionType.Copy,
                    scale=dinv_sb[:, n : n + 1],
                )
                nc.sync.dma_start(hs_shard[n * BLK : (n + 1) * BLK, :], hsa[:])

            # ---- AllGather the node table
            nc.gpsimd.collective_compute(
                "AllGather", mybir.AluOpType.bypass,
                replica_groups=[groups_all],
                ins=[hs_shard[:]], outs=[hs_full[:]],
            )

            # ---- message passing over chunks
            accum = apool.tile([BLK, nb, F], f32)
            ps_p = psppool.tile([1, F], f32)
            p_started = False
            psum_of_block = {}
            for ch, (q, entries) in enumerate(sched_chunks):
                idxt = wpool.tile([128, CHUNK // 16], i16, tag="idxt")
                nc.sync.dma_start(idxt[:], idx_e[ch])
                dstt = wpool.tile([128, CHUNK_GROUPS], f32, tag="dstt")
                nc.sync.dma_start(dstt[:], dst_e[ch])
                G = gpool.tile([128, CHUNK_GROUPS, F], bf16, tag="G")
                nc.gpsimd.dma_gather(
                    out_ap=G[:],
                    in_ap=hs_full[q * qrows : (q + 1) * qrows, :],
                    idxs_ap=idxt[:],
                    num_idxs=CHUNK, num_idxs_reg=CHUNK, elem_size=F,
                    single_packet=False, queue_num=(ch % NQUEUES),
                )
                for g, (b, flags) in enumerate(entries):
                    if b < 0:
                        continue
                    if flags["qstart"]:
                        psb = pspool.tile([BLK, F], f32, tag="ps")
                        psum_of_block[b] = psb
                        if flags["bias"]:
                            # psum := sqrt(deg) (x) b1  (K=1 outer product)
                            nc.tensor.matmul(
                                psb[:], sq_sb[:, b * BLK : (b + 1) * BLK],
                                b1_sb[:], start=True, stop=False,
                            )
                    psb = psum_of_block[b]
                    S = spool.tile([128, BLK], bf16, tag="S")
                    nc.vector.tensor_scalar(
                        S[:], iota_sb[:], dstt[:, g : g + 1], None,
                        op0=mybir.AluOpType.is_equal,
                    )
                    start = flags["qstart"] and not flags["bias"] and q == 0
                    # quarter>0 psum fresh accumulation also needs start=True
                    if flags["qstart"] and q > 0:
                        start = True
                    nc.tensor.matmul(
                        psb[:], S[:], G[:, g, :],
                        start=start, stop=flags["qend"],
                    )
                    if flags["qend"]:
                        del psum_of_block[b]
                        if q == 0:
                            nc.vector.tensor_copy(accum[:, b, :], psb[:])
                        elif not flags["final"]:
                            nc.vector.tensor_add(accum[:, b, :], accum[:, b, :], psb[:])
                        else:
                            nc.vector.tensor_add(accum[:, b, :], accum[:, b, :], psb[:])
                            h1b = spool.tile([BLK, F], f32, tag="h1b")
                            nc.scalar.activation(
                                h1b[:], accum[:, b, :],
                                mybir.ActivationFunctionType.Relu,
                                scale=dinv_sb[:, b : b + 1],
                            )
                            nc.tensor.matmul(
                                ps_p[:], u2_sb[:, b : b + 1], h1b[:],
                                start=not p_started, stop=(b == nb - 1),
                                skip_group_check=True,
                            )
                            p_started = True

            if DBG:
                for i in range(rows_total // 128):
                    hsdt = wpool.tile([128, F], bf16, tag="hsdt")
                    nc.sync.dma_start(hsdt[:], hs_full[i * 128 : (i + 1) * 128, :])
                    nc.sync.dma_start(hs_dbg[i * 128 : (i + 1) * 128, :], hsdt[:])
                nc.sync.dma_start(acc_dbg[:], accum[:])
            # ---- p AllReduce
            p_sb = cpool.tile([1, F], f32)
            nc.vector.tensor_copy(p_sb[:], ps_p[:])
            nc.sync.dma_start(p_dram[:], p_sb[:])
            if DBG:
                nc.sync.dma_start(ppre_dbg[:], p_sb[:])
            nc.gpsimd.collective_compute(
                "AllReduce", mybir.AluOpType.add,
                replica_groups=[groups_all],
                ins=[p_dram[:]], outs=[p_shared[:]],
            )
            # load p as a row, transpose to column on PE
            p_row = cpool.tile([1, F], f32)
            nc.sync.dma_start(p_row[:], p_shared[:])
            id1 = cpool.tile([1, 1], f32)
            nc.vector.memset(id1[:], 1.0)
            psT = pspool.tile([F, 1], f32, tag="ps")
            nc.tensor.transpose(psT[:], p_row[:], id1[:])
            p_col = cpool.tile([F, 1], f32)
            nc.vector.tensor_copy(p_col[:], psT[:])
            if DBG:
                nc.sync.dma_start(p_dbg[:], p_col[:])

            # ---- replicated tail MLP (column-vector chain on PE/ACT/DVE)
            tl = cpool
            def ld(e, shape, dt=f32):
                t = tl.tile(shape, dt, tag=f"c_{e.name}")
                nc.sync.dma_start(t[:], e[:])
                return t
            rates_sb = ld(rates_e, [16, 1]); encw1_sb = ld(encw1_e, [16, 8])
            encb1_sb = ld(encb1_e, [8, 1]); encw2_sb = ld(encw2_e, [8, F])
            encb2_sb = ld(encb2_e, [F, 1])
            w2a_sb = ld(w2a_e, [F, F]); w2b_sb = ld(w2b_e, [F, F])
            b2_sb = ld(b2_e, [F, 1]); s1_sb = ld(s1_e, [F, 1]); s2_sb = ld(s2_e, [F, 1])
            w3_sb = ld(w3_e, [F, F]); b3_sb = ld(b3_e, [F, 1])
            hidw_sb = ld(hidw_e, [F, 2 * F]); hidb_sb = ld(hidb_e, [F, 2])
            hid2wa_sb = ld(hid2wa_e, [F, F]); hid2wb_sb = ld(hid2wb_e, [F, F])
            hid2b_sb = ld(hid2b_e, [F, 1])
            finw_sb = ld(finw_e, [F, 2]); finb_sb = ld(finb_e, [2, 1])

            pst = pspool.tile([F, 2], f32, tag="ps")  # scratch psum, 2 cols

            # r1 = relu(encW1^T rates + encb1)   [8,1]
            nc.tensor.matmul(pst[:8, 0:1], encw1_sb[:], rates_sb[:], start=True, stop=True)
            r1 = tl.tile([8, 1], f32)
            nc.scalar.activation(r1[:], pst[:8, 0:1],
                                 mybir.ActivationFunctionType.Relu, bias=encb1_sb[:])
            # r2 = encW2^T r1 + encb2            [F,1]
            nc.tensor.matmul(pst[:, 1:2], encw2_sb[:], r1[:], start=True, stop=True)
            r2 = tl.tile([F, 1], f32)
            nc.vector.tensor_add(r2[:], pst[:, 1:2], encb2_sb[:])
            # m_r = S2 * r2
            mr = tl.tile([F, 1], f32)
            nc.vector.tensor_mul(mr[:], r2[:], s2_sb[:])
            # u1h2 = W2a^T p + W2b^T m_r + S1*b2 ; q = u1h2 / N
            pst2 = pspool.tile([F, 1], f32, tag="ps")
            nc.tensor.matmul(pst2[:], w2a_sb[:], p_col[:], start=True, stop=False)
            nc.tensor.matmul(pst2[:], w2b_sb[:], mr[:], start=False, stop=True)
            sb2 = tl.tile([F, 1], f32)
            nc.vector.tensor_mul(sb2[:], b2_sb[:], s1_sb[:])
            qv = tl.tile([F, 1], f32)
            nc.vector.tensor_add(qv[:], pst2[:], sb2[:])
            nc.vector.tensor_scalar_mul(qv[:], qv[:], 1.0 / dims["N"])
            # m3 = W3^T q + b3
            pst3 = pspool.tile([F, 1], f32, tag="ps")
            nc.tensor.matmul(pst3[:], w3_sb[:], qv[:], start=True, stop=True)
            m3 = tl.tile([F, 1], f32)
            nc.vector.tensor_add(m3[:], pst3[:], b3_sb[:])
            # g1 = relu(hidW^T m3 + hidb)  [256] as two cols
            g1a = tl.tile([F, 1], f32); g1b = tl.tile([F, 1], f32)
            nc.tensor.matmul(pst[:, 0:1], hidw_sb[:, :F], m3[:], start=True, stop=True)
            nc.scalar.activation(g1a[:], pst[:, 0:1],
                                 mybir.ActivationFunctionType.Relu, bias=hidb_sb[:, 0:1])
            nc.tensor.matmul(pst[:, 1:2], hidw_sb[:, F:], m3[:], start=True, stop=True)
            nc.scalar.activation(g1b[:], pst[:, 1:2],
                                 mybir.ActivationFunctionType.Relu, bias=hidb_sb[:, 1:2])
            # g2 = relu(hid2W^T g1 + hid2b)  [F,1]
            pst4 = pspool.tile([F, 1], f32, tag="ps")
            nc.tensor.matmul(pst4[:], hid2wa_sb[:], g1a[:], start=True, stop=False)
            nc.tensor.matmul(pst4[:], hid2wb_sb[:], g1b[:], start=False, stop=True)
            g2 = tl.tile([F, 1], f32)
            nc.scalar.activation(g2[:], pst4[:],
                                 mybir.ActivationFunctionType.Relu, bias=hid2b_sb[:])
            # out = finW^T g2 + finb  [2,1]
            pst5 = pspool.tile([2, 1], f32, tag="ps")
            nc.tensor.matmul(pst5[:], finw_sb[:], g2[:], start=True, stop=True)
            outv = tl.tile([2, 1], f32)
            nc.vector.tensor_add(outv[:], pst5[:], finb_sb[:])
            nc.sync.dma_start(out_e[:], outv[:])

    nc.compile()
    return nc


_CACHE = {}
LAST_RESULTS = None


def kernel(**inputs):
    graph = np.asarray(inputs["graph"], np.float32)
    edge_index = np.asarray(inputs["edge_index"], np.int64)
    rates = np.asarray(inputs["rates"], np.float32)
    params = {k: np.asarray(v) for k, v in inputs.items()
              if k not in ("graph", "edge_index", "rates")}
    in_maps, sched, dims = _preprocess(graph, edge_index, rates, params)
    key = (dims["nchunk"], dims["shard_pad"],
           tuple((q, tuple(b for b, _ in e)) for q, e in sched))
    if key not in _CACHE:
        _CACHE[key] = _build(sched, dims)
    nc = _CACHE[key]
    import os
    trace = bool(int(os.environ.get("GCN_TRACE", "0")))
    res = run_bass_kernel_spmd(nc, in_maps, list(range(N_CORES)), trace=trace)
    global LAST_RESULTS
    LAST_RESULTS = res
    out = np.asarray(res.results[0]["out"], np.float32).reshape(1, 2)
    return out



# revision 11
# speedup vs baseline: 1.5159x; 1.5159x over previous
"""GCN (3-layer + MLP head) on 8 Trainium2 NeuronCores.

Strategy (graph-parallel, dst-sharded, SWDGE-gather message passing):
  - Host renumbers nodes (LPT bin-packing on in-degree) so every 128-node
    dst block has total in-degree <= 2048; nodes shard 8 ways by new id.
  - Phase A (per core): Hs = dinv * (X_shard @ W1) as bf16 rows, kept in
    SBUF (self-loop term) and AllGathered -> full node table in HBM.
  - MP phase: each dst block's edges are sorted by src table row and split
    into 8 segments of exactly 256 edges (last = remainder); segment k of
    all blocks lies in a fixed ~32k-row window, so int16 gather indices
    cover it with zero per-cell padding.  Chunks of 16 blocks' k-segments
    (4096 idxs) feed dma_gather; per 128-edge group a one-hot S matrix
    (DVE is_equal vs iota) scatter-sums messages into the dst block via a
    PE matmul; segments accumulate into an SBUF f32 accum.
  - Self loops never enter the gather: an identity-matrix matmul adds the
    core's own Hs rows into each block's psum at segment 0.
  - GCN layers 2+3 feed only a global mean, so they collapse to weighted
    node sums (host-precomputed u2, S1, S2); p = sum_d u2[d]*relu(h1[d])
    is a PE reduction, AllReduced across cores; tiny MLP head replicated.
"""
import heapq
import numpy as np
import ml_dtypes

import concourse.bass as bass
import concourse.tile as tile
from concourse import bacc, mybir
from concourse.bass_utils import run_bass_kernel_spmd

N_CORES = 8
N = 100000
F = 128            # feature dim
BLK = 128          # dst-block size (psum partition dim)
NB = 98            # blocks per core
SHARD = NB * BLK   # 12544 rows per core (incl. pad rows)
ROWS = N_CORES * SHARD  # 100352 table rows
NSEG = 8           # segments per block
SEG = 256          # edges per segment (2 matmul groups)
GPB = NSEG * 2     # groups per block = 16
CPP = 7            # chunks per pass: 6 full (16 blocks) + 1 tail (2 blocks)
NCHUNK = NSEG * CPP
CH_FULL = 16 * SEG  # 4096 idxs
CH_TAIL = 2 * SEG   # 512 idxs
WIN = 32768
NQUEUES = 4
PAD_DST = 130.0

BF16 = ml_dtypes.bfloat16


def _win_base(k):
    return max(0, min(k * SHARD - 3584, ROWS - WIN))


# ----------------------------------------------------------------------------
# host preprocessing: renumber, normalize, sort, segment
# ----------------------------------------------------------------------------
def _preprocess(graph, edge_index, rates, params):
    src = np.asarray(edge_index[0], np.int64)
    dst = np.asarray(edge_index[1], np.int64)
    E = src.shape[0]

    # normalization scalars (f64, original ids; order-independent)
    deg = np.bincount(dst, minlength=N).astype(np.float64) + 1.0
    dinv = deg ** -0.5
    sq = deg ** 0.5
    u1 = dinv * (np.bincount(src, weights=dinv[dst], minlength=N) + dinv)
    y = u1 * dinv
    u2 = dinv * (np.bincount(src, weights=y[dst], minlength=N) + y)
    S1 = float(u1.sum())
    S2 = float(u2.sum())

    # LPT renumbering: balance per-block in-degree (target <= 2048)
    indeg = np.bincount(dst, minlength=N)
    nbins = N_CORES * NB
    order = np.argsort(-indeg, kind="stable")
    heap = [(0, 0, b) for b in range(nbins)]
    heapq.heapify(heap)
    binof = np.empty(N, np.int32)
    posof = np.empty(N, np.int32)
    for v in order:
        load, cnt, b = heapq.heappop(heap)
        binof[v] = b
        posof[v] = cnt
        cnt += 1
        load += int(indeg[v])
        if cnt < BLK:
            heapq.heappush(heap, (load, cnt, b))
    newid = binof.astype(np.int64) * BLK + posof
    # table row order is partition-major per core: row = c*SHARD + p*NB + b,
    # so the SBUF hs tile [128, NB, F] stores to DRAM as a single identity DMA
    c_ = newid // SHARD
    b_ = (newid % SHARD) // BLK
    p_ = newid % BLK
    trow = c_ * SHARD + p_ * NB + b_

    ns, nd = trow[src], newid[dst]
    cell = nd // BLK                      # global cell id = core*NB + block
    o = np.lexsort((ns, cell))
    cell_s, ns_s, off_s = cell[o], ns[o], (nd[o] % BLK).astype(np.float64)
    ccnt = np.bincount(cell_s, minlength=nbins)
    assert ccnt.max() <= NSEG * SEG, f"cell overflow {ccnt.max()}"
    cstart = np.zeros(nbins + 1, np.int64)
    cstart[1:] = np.cumsum(ccnt)

    # per-core edge tables in schedule order
    idx16 = np.zeros((N_CORES, NCHUNK, 128, CH_FULL // 16), np.int16)
    dstid = np.full((N_CORES, NCHUNK, 128, CH_FULL // 128), PAD_DST, np.float32)
    iv = np.zeros(CH_FULL, np.int64)
    ov = np.zeros(CH_FULL, np.float64)
    for c in range(N_CORES):
        for k in range(NSEG):
            base = _win_base(k)
            for bg in range(CPP):
                blo, bhi = bg * 16, min(bg * 16 + 16, NB)
                ncell = bhi - blo
                chn = ncell * SEG
                iv[:chn] = 0
                ov[:chn] = PAD_DST
                for ci2 in range(ncell):
                    g_cell = c * NB + blo + ci2
                    s0, T = cstart[g_cell], ccnt[g_cell]
                    a, b2 = k * SEG, min((k + 1) * SEG, T)
                    if a >= b2:
                        continue
                    take = b2 - a
                    rel = ns_s[s0 + a : s0 + b2] - base
                    assert rel.min() >= 0 and rel.max() < WIN, (c, k, bg, ci2)
                    p0 = ci2 * SEG
                    iv[p0 : p0 + take] = rel
                    ov[p0 : p0 + take] = off_s[s0 + a : s0 + b2]
                ci = k * CPP + bg
                e = np.arange(chn)
                tmp = np.zeros((16, CH_FULL // 16), np.int16)
                tmp[e % 16, e // 16] = iv[:chn]
                idx16[c, ci] = np.tile(tmp, (8, 1))
                dstid[c, ci][e % 128, e // 128] = ov[:chn].astype(np.float32)

    # phase A inputs (new ordering, padded)
    X = np.asarray(graph, np.float32)
    inv = np.full(ROWS, -1, np.int64)
    inv[newid] = np.arange(N)
    xt = np.zeros((N_CORES, F, SHARD), np.float32)
    dinv_pm = np.zeros((N_CORES, BLK, NB), np.float32)
    u2_pm = np.zeros((N_CORES, BLK, NB), np.float32)
    sqdeg = np.zeros((N_CORES, 1, SHARD), np.float32)
    dv = np.zeros(ROWS)
    uv = np.zeros(ROWS)
    sv = np.zeros(ROWS)
    real = inv >= 0
    dv[real] = dinv[inv[real]]
    uv[real] = u2[inv[real]]
    sv[real] = sq[inv[real]]
    for c in range(N_CORES):
        rows = inv[c * SHARD : (c + 1) * SHARD]
        m = rows >= 0
        xt[c][:, m] = X[rows[m]].T
        dinv_pm[c] = dv[c * SHARD : (c + 1) * SHARD].reshape(NB, BLK).T
        u2_pm[c] = uv[c * SHARD : (c + 1) * SHARD].reshape(NB, BLK).T
        sqdeg[c, 0] = sv[c * SHARD : (c + 1) * SHARD]

    p = params
    col = lambda v: np.asarray(v, np.float32).reshape(-1, 1)
    iota = np.tile(np.arange(BLK, dtype=np.float32)[None, :], (128, 1))
    ident = np.eye(128, dtype=BF16)
    common = dict(
        w1=np.asarray(p["conv1_W"], np.float32),
        b1row=np.asarray(p["conv1_b"], BF16).reshape(1, F),
        iota=iota,
        ident=ident,
        rates_col=col(rates),
        encw1=np.asarray(p["enc_W1"], np.float32),
        encb1=col(p["enc_b1"]),
        encw2=np.asarray(p["enc_W2"], np.float32),
        encb2=col(p["enc_b2"]),
        w2a=np.asarray(p["conv2_W"], np.float32)[:F],
        w2b=np.asarray(p["conv2_W"], np.float32)[F:],
        b2col=col(p["conv2_b"]),
        s1col=np.full((F, 1), S1, np.float32),
        s2col=np.full((F, 1), S2, np.float32),
        w3=np.asarray(p["conv3_W"], np.float32),
        b3col=col(p["conv3_b"]),
        hidw=np.asarray(p["hid_W"], np.float32),
        hidb=np.asarray(p["hid_b"], np.float32).reshape(2, F).T,
        hid2wa=np.asarray(p["hid2_W"], np.float32)[:F],
        hid2wb=np.asarray(p["hid2_W"], np.float32)[F:],
        hid2b=col(p["hid2_b"]),
        finw=np.asarray(p["fin_W"], np.float32),
        finb=col(p["fin_b"]),
    )
    in_maps = []
    for c in range(N_CORES):
        m = dict(common)
        m.update(
            xt=xt[c], sqdeg=sqdeg[c].astype(BF16), dinv=dinv_pm[c],
            u2c=u2_pm[c], idx16=idx16[c], dstid=dstid[c],
        )
        in_maps.append(m)
    return in_maps


# ----------------------------------------------------------------------------
# device program
# ----------------------------------------------------------------------------
def _build():
    f32, bf16, i16 = mybir.dt.float32, mybir.dt.bfloat16, mybir.dt.int16

    nc = bacc.Bacc("TRN2", target_bir_lowering=False, debug=False,
                   num_devices=N_CORES, num_swdge_queues=NQUEUES)
    I = lambda name, shape, dt=f32: nc.dram_tensor(name, shape, dt, kind="ExternalInput")
    xt_e = I("xt", [F, SHARD])
    w1_e = I("w1", [F, F]); b1_e = I("b1row", [1, F], bf16)
    sq_e = I("sqdeg", [1, SHARD], bf16)
    dinv_e = I("dinv", [BLK, NB]); u2_e = I("u2c", [BLK, NB])
    idx_e = I("idx16", [NCHUNK, 128, CH_FULL // 16], i16)
    dst_e = I("dstid", [NCHUNK, 128, CH_FULL // 128])
    iota_e = I("iota", [128, BLK])
    ident_e = I("ident", [128, 128], bf16)
    rates_e = I("rates_col", [16, 1])
    encw1_e = I("encw1", [16, 8]); encb1_e = I("encb1", [8, 1])
    encw2_e = I("encw2", [8, F]); encb2_e = I("encb2", [F, 1])
    w2a_e = I("w2a", [F, F]); w2b_e = I("w2b", [F, F]); b2_e = I("b2col", [F, 1])
    s1_e = I("s1col", [F, 1]); s2_e = I("s2col", [F, 1])
    w3_e = I("w3", [F, F]); b3_e = I("b3col", [F, 1])
    hidw_e = I("hidw", [F, 2 * F]); hidb_e = I("hidb", [F, 2])
    hid2wa_e = I("hid2wa", [F, F]); hid2wb_e = I("hid2wb", [F, F])
    hid2b_e = I("hid2b", [F, 1])
    finw_e = I("finw", [F, 2]); finb_e = I("finb", [2, 1])
    out_e = nc.dram_tensor("out", [2, 1], f32, kind="ExternalOutput")

    hs_shard = nc.dram_tensor("hs_shard", [128, NB * F], bf16)
    hs_full = nc.dram_tensor("hs_full", [ROWS, F], bf16, addr_space="Shared")
    p_dram = nc.dram_tensor("p_dram", [1, F], f32)
    p_shared = nc.dram_tensor("p_shared", [1, F], f32, addr_space="Shared")
    groups_all = list(range(N_CORES))

    with tile.TileContext(nc) as tc:
        with (
            tc.tile_pool(name="const", bufs=1) as cpool,
            tc.tile_pool(name="xt", bufs=3) as xtpool,
            tc.tile_pool(name="hself", bufs=1) as hpool,
            tc.tile_pool(name="work", bufs=8) as wpool,
            tc.tile_pool(name="gat", bufs=6) as gpool,
            tc.tile_pool(name="sstile", bufs=4) as spool,
            tc.tile_pool(name="accum", bufs=1) as apool,
            tc.tile_pool(name="ps", bufs=4, space="PSUM") as pspool,
            tc.tile_pool(name="psp", bufs=1, space="PSUM") as psppool,
        ):
            # ---- constants
            w1_sb = cpool.tile([F, F], f32); nc.sync.dma_start(w1_sb[:], w1_e[:])
            b1_sb = cpool.tile([1, F], bf16); nc.sync.dma_start(b1_sb[:], b1_e[:])
            sq_sb = cpool.tile([1, SHARD], bf16); nc.sync.dma_start(sq_sb[:], sq_e[:])
            dinv_sb = cpool.tile([BLK, NB], f32); nc.sync.dma_start(dinv_sb[:], dinv_e[:])
            u2_sb = cpool.tile([BLK, NB], f32); nc.sync.dma_start(u2_sb[:], u2_e[:])
            iota_sb = cpool.tile([128, BLK], f32); nc.sync.dma_start(iota_sb[:], iota_e[:])
            ident_sb = cpool.tile([128, 128], bf16); nc.sync.dma_start(ident_sb[:], ident_e[:])

            # ---- phase A: Hs = dinv * (X @ W1), bf16, kept in SBUF
            hs_self = hpool.tile([128, NB, F], bf16)
            for i in range(14):
                xts = xtpool.tile([F, 7 * BLK], f32, tag="xts")
                nc.sync.dma_start(xts[:], xt_e[:, i * 7 * BLK : (i + 1) * 7 * BLK])
                for j in range(7):
                    b = i * 7 + j
                    psA = pspool.tile([BLK, F], f32, tag="ps")
                    nc.tensor.matmul(psA[:], xts[:, j * BLK : (j + 1) * BLK],
                                     w1_sb[:], start=True, stop=True)
                    nc.scalar.activation(
                        hs_self[:, b, :], psA[:], mybir.ActivationFunctionType.Copy,
                        scale=dinv_sb[:, b : b + 1],
                    )
            # store shard to DRAM for the AllGather (identity layout, one DMA)
            nc.sync.dma_start(hs_shard[:], hs_self[:])

            # ---- AllGather the node table
            nc.gpsimd.collective_compute(
                "AllGather", mybir.AluOpType.bypass,
                replica_groups=[groups_all],
                ins=[hs_shard[:]], outs=[hs_full[:]],
            )

            # ---- message passing: 8 segment passes x 7 chunks
            accum = apool.tile([BLK, NB, F], f32)
            ps_p = psppool.tile([1, F], f32)
            for k in range(NSEG):
                base = _win_base(k)
                for bg in range(CPP):
                    ci = k * CPP + bg
                    blo = bg * 16
                    ncell = min(16, NB - blo)
                    chn = ncell * SEG
                    idxt = wpool.tile([128, CH_FULL // 16], i16, tag="idxt")
                    nc.sync.dma_start(idxt[:, : chn // 16], idx_e[ci, :, : chn // 16])
                    dstt = wpool.tile([128, CH_FULL // 128], f32, tag="dstt")
                    nc.sync.dma_start(dstt[:, : chn // 128], dst_e[ci, :, : chn // 128])
                    G = gpool.tile([128, CH_FULL // 128, F], bf16, tag="G")
                    nc.gpsimd.dma_gather(
                        out_ap=G[:, : chn // 128, :],
                        in_ap=hs_full[base : base + WIN, :],
                        idxs_ap=idxt[:, : chn // 16],
                        num_idxs=chn, num_idxs_reg=chn, elem_size=F,
                        single_packet=False, queue_num=(ci % NQUEUES),
                    )
                    for ci2 in range(ncell):
                        b = blo + ci2
                        psb = pspool.tile([BLK, F], f32, tag="ps")
                        if k == 0:
                            # bias (sqrt(deg) (x) b1) + self-loop rows
                            nc.tensor.matmul(
                                psb[:], sq_sb[:, b * BLK : (b + 1) * BLK],
                                b1_sb[:], start=True, stop=False,
                            )
                            nc.tensor.matmul(
                                psb[:], ident_sb[:], hs_self[:, b, :],
                                start=False, stop=False,
                            )
                        for g2 in range(2):
                            g = ci2 * 2 + g2
                            S = spool.tile([128, BLK], bf16, tag="S")
                            nc.vector.tensor_scalar(
                                S[:], iota_sb[:], dstt[:, g : g + 1], None,
                                op0=mybir.AluOpType.is_equal,
                            )
                            nc.tensor.matmul(
                                psb[:], S[:], G[:, g, :],
                                start=(k > 0 and g2 == 0), stop=(g2 == 1),
                            )
                        if k == 0:
                            nc.vector.tensor_copy(accum[:, b, :], psb[:])
                        elif k < NSEG - 1:
                            nc.vector.tensor_add(accum[:, b, :], accum[:, b, :], psb[:])
                        else:
                            nc.vector.tensor_add(accum[:, b, :], accum[:, b, :], psb[:])
                            h1b = spool.tile([BLK, F], f32, tag="h1b")
                            nc.scalar.activation(
                                h1b[:], accum[:, b, :],
                                mybir.ActivationFunctionType.Relu,
                                scale=dinv_sb[:, b : b + 1],
                            )
                            nc.tensor.matmul(
                                ps_p[:], u2_sb[:, b : b + 1], h1b[:],
                                start=(b == 0), stop=(b == NB - 1),
                                skip_group_check=True,
                            )

            # ---- p AllReduce
            p_sb = cpool.tile([1, F], f32)
            nc.vector.tensor_copy(p_sb[:], ps_p[:])
            nc.sync.dma_start(p_dram[:], p_sb[:])
            nc.gpsimd.collective_compute(
                "AllReduce", mybir.AluOpType.add,
                replica_groups=[groups_all],
                ins=[p_dram[:]], outs=[p_shared[:]],
            )
            p_row = cpool.tile([1, F], f32)
            nc.sync.dma_start(p_row[:], p_shared[:])
            id1 = cpool.tile([1, 1], f32)
            nc.vector.memset(id1[:], 1.0)
            psT = pspool.tile([F, 1], f32, tag="ps")
            nc.tensor.transpose(psT[:], p_row[:], id1[:])
            p_col = cpool.tile([F, 1], f32)
            nc.vector.tensor_copy(p_col[:], psT[:])

            # ---- replicated tail MLP
            tl = cpool
            def ld(e, shape, dt=f32):
                t = tl.tile(shape, dt, tag=f"c_{e.name}")
                nc.sync.dma_start(t[:], e[:])
                return t
            rates_sb = ld(rates_e, [16, 1]); encw1_sb = ld(encw1_e, [16, 8])
            encb1_sb = ld(encb1_e, [8, 1]); encw2_sb = ld(encw2_e, [8, F])
            encb2_sb = ld(encb2_e, [F, 1])
            w2a_sb = ld(w2a_e, [F, F]); w2b_sb = ld(w2b_e, [F, F])
            b2_sb = ld(b2_e, [F, 1]); s1_sb = ld(s1_e, [F, 1]); s2_sb = ld(s2_e, [F, 1])
            w3_sb = ld(w3_e, [F, F]); b3_sb = ld(b3_e, [F, 1])
            hidw_sb = ld(hidw_e, [F, 2 * F]); hidb_sb = ld(hidb_e, [F, 2])
            hid2wa_sb = ld(hid2wa_e, [F, F]); hid2wb_sb = ld(hid2wb_e, [F, F])
            hid2b_sb = ld(hid2b_e, [F, 1])
            finw_sb = ld(finw_e, [F, 2]); finb_sb = ld(finb_e, [2, 1])

            pst = pspool.tile([F, 2], f32, tag="ps")
            nc.tensor.matmul(pst[:8, 0:1], encw1_sb[:], rates_sb[:], start=True, stop=True)
            r1 = tl.tile([8, 1], f32)
            nc.scalar.activation(r1[:], pst[:8, 0:1],
                                 mybir.ActivationFunctionType.Relu, bias=encb1_sb[:])
            nc.tensor.matmul(pst[:, 1:2], encw2_sb[:], r1[:], start=True, stop=True)
            r2 = tl.tile([F, 1], f32)
            nc.vector.tensor_add(r2[:], pst[:, 1:2], encb2_sb[:])
            mr = tl.tile([F, 1], f32)
            nc.vector.tensor_mul(mr[:], r2[:], s2_sb[:])
            pst2 = pspool.tile([F, 1], f32, tag="ps")
            nc.tensor.matmul(pst2[:], w2a_sb[:], p_col[:], start=True, stop=False)
            nc.tensor.matmul(pst2[:], w2b_sb[:], mr[:], start=False, stop=True)
            sb2 = tl.tile([F, 1], f32)
            nc.vector.tensor_mul(sb2[:], b2_sb[:], s1_sb[:])
            qv = tl.tile([F, 1], f32)
            nc.vector.tensor_add(qv[:], pst2[:], sb2[:])
            nc.vector.tensor_scalar_mul(qv[:], qv[:], 1.0 / N)
            pst3 = pspool.tile([F, 1], f32, tag="ps")
            nc.tensor.matmul(pst3[:], w3_sb[:], qv[:], start=True, stop=True)
            m3 = tl.tile([F, 1], f32)
            nc.vector.tensor_add(m3[:], pst3[:], b3_sb[:])
            g1a = tl.tile([F, 1], f32); g1b = tl.tile([F, 1], f32)
            nc.tensor.matmul(pst[:, 0:1], hidw_sb[:, :F], m3[:], start=True, stop=True)
            nc.scalar.activation(g1a[:], pst[:, 0:1],
                                 mybir.ActivationFunctionType.Relu, bias=hidb_sb[:, 0:1])
            nc.tensor.matmul(pst[:, 1:2], hidw_sb[:, F:], m3[:], start=True, stop=True)
            nc.scalar.activation(g1b[:], pst[:, 1:2],
                                 mybir.ActivationFunctionType.Relu, bias=hidb_sb[:, 1:2])
            pst4 = pspool.tile([F, 1], f32, tag="ps")
            nc.tensor.matmul(pst4[:], hid2wa_sb[:], g1a[:], start=True, stop=False)
            nc.tensor.matmul(pst4[:], hid2wb_sb[:], g1b[:], start=False, stop=True)
            g2 = tl.tile([F, 1], f32)
            nc.scalar.activation(g2[:], pst4[:],
                                 mybir.ActivationFunctionType.Relu, bias=hid2b_sb[:])
            pst5 = pspool.tile([2, 1], f32, tag="ps")
            nc.tensor.matmul(pst5[:], finw_sb[:], g2[:], start=True, stop=True)
            outv = tl.tile([2, 1], f32)
            nc.vector.tensor_add(outv[:], pst5[:], finb_sb[:])
            nc.sync.dma_start(out_e[:], outv[:])

    nc.compile()
    return nc


_CACHE = {}
LAST_RESULTS = None


def kernel(**inputs):
    graph = np.asarray(inputs["graph"], np.float32)
    edge_index = np.asarray(inputs["edge_index"], np.int64)
    rates = np.asarray(inputs["rates"], np.float32)
    params = {k: np.asarray(v) for k, v in inputs.items()
              if k not in ("graph", "edge_index", "rates")}
    in_maps = _preprocess(graph, edge_index, rates, params)
    if "nc" not in _CACHE:
        _CACHE["nc"] = _build()
    nc = _CACHE["nc"]
    import os
    trace = bool(int(os.environ.get("GCN_TRACE", "0")))
    res = run_bass_kernel_spmd(nc, in_maps, list(range(N_CORES)), trace=trace)
    global LAST_RESULTS
    LAST_RESULTS = res
    out = np.asarray(res.results[0]["out"], np.float32).reshape(1, 2)
    return out


# revision 24
# speedup vs baseline: 1.6220x; 1.0700x over previous
"""GCN (3-layer + MLP head) on 8 Trainium2 NeuronCores.

Strategy (graph-parallel, dst-sharded, SWDGE-gather message passing):
  - Host renumbers nodes (LPT bin-packing on in-degree) so every 128-node
    dst block has total in-degree <= 2048; nodes shard 8 ways by new id.
  - Phase A (per core): Hs = dinv * (X_shard @ W1) as bf16 rows, kept in
    SBUF (self-loop term) and AllGathered -> full node table in HBM.
  - MP phase: each dst block's edges are sorted by src table row and split
    into 8 segments of exactly 256 edges (last = remainder); segment k of
    all blocks lies in a fixed ~32k-row window, so int16 gather indices
    cover it with zero per-cell padding.  Chunks of 16 blocks' k-segments
    (4096 idxs) feed dma_gather; per 128-edge group a one-hot S matrix
    (DVE is_equal vs iota) scatter-sums messages into the dst block via a
    PE matmul; segments accumulate into an SBUF f32 accum.
  - Self loops never enter the gather: an identity-matrix matmul adds the
    core's own Hs rows into each block's psum at segment 0.
  - GCN layers 2+3 feed only a global mean, so they collapse to weighted
    node sums (host-precomputed u2, S1, S2); p = sum_d u2[d]*relu(h1[d])
    is a PE reduction, AllReduced across cores; tiny MLP head replicated.
"""
import heapq
import numpy as np
import ml_dtypes

import concourse.bass as bass
import concourse.tile as tile
from concourse import bacc, mybir
from concourse.bass_utils import run_bass_kernel_spmd

N_CORES = 8
N = 100000
F = 128            # feature dim
BLK = 128          # dst-block size (psum partition dim)
NB = 98            # blocks per core
SHARD = NB * BLK   # 12544 rows per core (incl. pad rows)
ROWS = N_CORES * SHARD  # 100352 table rows
NSEG = 8           # segments per block
SEG = 256          # edges per segment (2 matmul groups)
GPB = NSEG * 2     # groups per block = 16
CPP = 7            # chunks per pass: 6 full (16 blocks) + 1 tail (2 blocks)
NCHUNK = NSEG * CPP
CH_FULL = 16 * SEG  # 4096 idxs
CH_TAIL = 2 * SEG   # 512 idxs
WIN = 32768
NQUEUES = 4
PAD_DST = 130.0

BF16 = ml_dtypes.bfloat16


def _win_base(k):
    return max(0, min(k * SHARD - 3584, ROWS - WIN))


# ----------------------------------------------------------------------------
# host preprocessing: renumber, normalize, sort, segment
# ----------------------------------------------------------------------------
def _preprocess(graph, edge_index, rates, params):
    src = np.asarray(edge_index[0], np.int64)
    dst = np.asarray(edge_index[1], np.int64)
    E = src.shape[0]

    # normalization scalars (f64, original ids; order-independent)
    deg = np.bincount(dst, minlength=N).astype(np.float64) + 1.0
    dinv = deg ** -0.5
    sq = deg ** 0.5
    u1 = dinv * (np.bincount(src, weights=dinv[dst], minlength=N) + dinv)
    y = u1 * dinv
    u2 = dinv * (np.bincount(src, weights=y[dst], minlength=N) + y)
    S1 = float(u1.sum())
    S2 = float(u2.sum())

    # LPT renumbering: balance per-block in-degree (target <= 2048)
    indeg = np.bincount(dst, minlength=N)
    nbins = N_CORES * NB
    order = np.argsort(-indeg, kind="stable")
    heap = [(0, 0, b) for b in range(nbins)]
    heapq.heapify(heap)
    binof = np.empty(N, np.int32)
    posof = np.empty(N, np.int32)
    for v in order:
        load, cnt, b = heapq.heappop(heap)
        binof[v] = b
        posof[v] = cnt
        cnt += 1
        load += int(indeg[v])
        if cnt < BLK:
            heapq.heappush(heap, (load, cnt, b))
    newid = binof.astype(np.int64) * BLK + posof
    # table row order is partition-major per core: row = c*SHARD + p*NB + b,
    # so the SBUF hs tile [128, NB, F] stores to DRAM as a single identity DMA
    c_ = newid // SHARD
    b_ = (newid % SHARD) // BLK
    p_ = newid % BLK
    trow = c_ * SHARD + p_ * NB + b_

    ns, nd = trow[src], newid[dst]
    cell = nd // BLK                      # global cell id = core*NB + block
    o = np.lexsort((ns, cell))
    cell_s, ns_s, off_s = cell[o], ns[o], (nd[o] % BLK).astype(np.float64)
    ccnt = np.bincount(cell_s, minlength=nbins)
    assert ccnt.max() <= NSEG * SEG, f"cell overflow {ccnt.max()}"
    cstart = np.zeros(nbins + 1, np.int64)
    cstart[1:] = np.cumsum(ccnt)

    # per-core edge tables in schedule order (block-group-major: ci = bg*8+k)
    idx16 = np.zeros((N_CORES, NCHUNK, 128, CH_FULL // 16), np.int16)
    dstid = np.full((N_CORES, NCHUNK, 128, CH_FULL // 128), PAD_DST, np.float32)
    iv = np.zeros(CH_FULL, np.int64)
    ov = np.zeros(CH_FULL, np.float64)
    for c in range(N_CORES):
        for bg in range(CPP):
            blo, bhi = bg * 16, min(bg * 16 + 16, NB)
            ncell = bhi - blo
            chn = ncell * SEG
            for k in range(NSEG):
                base = _win_base(k)
                iv[:chn] = 0
                ov[:chn] = PAD_DST
                for ci2 in range(ncell):
                    g_cell = c * NB + blo + ci2
                    s0, T = cstart[g_cell], ccnt[g_cell]
                    a, b2 = k * SEG, min((k + 1) * SEG, T)
                    if a >= b2:
                        continue
                    take = b2 - a
                    rel = ns_s[s0 + a : s0 + b2] - base
                    assert rel.min() >= 0 and rel.max() < WIN, (c, k, bg, ci2)
                    p0 = ci2 * SEG
                    iv[p0 : p0 + take] = rel
                    ov[p0 : p0 + take] = off_s[s0 + a : s0 + b2]
                ci = bg * NSEG + k
                e = np.arange(chn)
                tmp = np.zeros((16, CH_FULL // 16), np.int16)
                tmp[e % 16, e // 16] = iv[:chn]
                idx16[c, ci] = np.tile(tmp, (8, 1))
                dstid[c, ci][e % 128, e // 128] = ov[:chn].astype(np.float32)

    # phase A inputs (new ordering, padded)
    X = np.asarray(graph, np.float32)
    inv = np.full(ROWS, -1, np.int64)
    inv[newid] = np.arange(N)
    xt = np.zeros((N_CORES, F, SHARD), np.float32)
    dinv_pm = np.zeros((N_CORES, BLK, NB), np.float32)
    u2_pm = np.zeros((N_CORES, BLK, NB), np.float32)
    sqdeg = np.zeros((N_CORES, 1, SHARD), np.float32)
    dv = np.zeros(ROWS)
    uv = np.zeros(ROWS)
    sv = np.zeros(ROWS)
    real = inv >= 0
    dv[real] = dinv[inv[real]]
    uv[real] = u2[inv[real]]
    sv[real] = sq[inv[real]]
    for c in range(N_CORES):
        rows = inv[c * SHARD : (c + 1) * SHARD]
        m = rows >= 0
        xt[c][:, m] = X[rows[m]].T
        dinv_pm[c] = dv[c * SHARD : (c + 1) * SHARD].reshape(NB, BLK).T
        u2_pm[c] = uv[c * SHARD : (c + 1) * SHARD].reshape(NB, BLK).T
        sqdeg[c, 0] = sv[c * SHARD : (c + 1) * SHARD]

    p = params
    col = lambda v: np.asarray(v, np.float32).reshape(-1, 1)
    iota = np.tile(np.arange(BLK, dtype=BF16)[None, :], (128, 1))
    ident = np.eye(128, dtype=BF16)
    common = dict(
        w1=np.asarray(p["conv1_W"], np.float32),
        b1row=np.asarray(p["conv1_b"], BF16).reshape(1, F),
        iota=iota,
        ident=ident,
        rates_col=col(rates),
        encw1=np.asarray(p["enc_W1"], np.float32),
        encb1=col(p["enc_b1"]),
        encw2=np.asarray(p["enc_W2"], np.float32),
        encb2=col(p["enc_b2"]),
        w2a=np.asarray(p["conv2_W"], np.float32)[:F],
        w2b=np.asarray(p["conv2_W"], np.float32)[F:],
        b2col=col(p["conv2_b"]),
        s1col=np.full((F, 1), S1, np.float32),
        s2col=np.full((F, 1), S2, np.float32),
        w3=np.asarray(p["conv3_W"], np.float32),
        b3col=col(p["conv3_b"]),
        hidw=np.asarray(p["hid_W"], np.float32),
        hidb=np.asarray(p["hid_b"], np.float32).reshape(2, F).T,
        hid2wa=np.asarray(p["hid2_W"], np.float32)[:F],
        hid2wb=np.asarray(p["hid2_W"], np.float32)[F:],
        hid2b=col(p["hid2_b"]),
        finw=np.asarray(p["fin_W"], np.float32),
        finb=col(p["fin_b"]),
    )
    in_maps = []
    for c in range(N_CORES):
        m = dict(common)
        m.update(
            xt=xt[c], sqdeg=sqdeg[c].astype(BF16), dinv=dinv_pm[c],
            u2c=u2_pm[c], idx16=idx16[c],
            dstid=np.ascontiguousarray(dstid[c].transpose(1, 0, 2)),
        )
        in_maps.append(m)
    return in_maps


# ----------------------------------------------------------------------------
# device program
# ----------------------------------------------------------------------------
def _build():
    f32, bf16, i16 = mybir.dt.float32, mybir.dt.bfloat16, mybir.dt.int16

    nc = bacc.Bacc("TRN2", target_bir_lowering=False, debug=False,
                   num_devices=N_CORES, num_swdge_queues=NQUEUES)
    I = lambda name, shape, dt=f32: nc.dram_tensor(name, shape, dt, kind="ExternalInput")
    xt_e = I("xt", [F, SHARD])
    w1_e = I("w1", [F, F]); b1_e = I("b1row", [1, F], bf16)
    sq_e = I("sqdeg", [1, SHARD], bf16)
    dinv_e = I("dinv", [BLK, NB]); u2_e = I("u2c", [BLK, NB])
    idx_e = I("idx16", [NCHUNK, 128, CH_FULL // 16], i16)
    dst_e = I("dstid", [128, NCHUNK, CH_FULL // 128])
    iota_e = I("iota", [128, BLK], bf16)
    ident_e = I("ident", [128, 128], bf16)
    rates_e = I("rates_col", [16, 1])
    encw1_e = I("encw1", [16, 8]); encb1_e = I("encb1", [8, 1])
    encw2_e = I("encw2", [8, F]); encb2_e = I("encb2", [F, 1])
    w2a_e = I("w2a", [F, F]); w2b_e = I("w2b", [F, F]); b2_e = I("b2col", [F, 1])
    s1_e = I("s1col", [F, 1]); s2_e = I("s2col", [F, 1])
    w3_e = I("w3", [F, F]); b3_e = I("b3col", [F, 1])
    hidw_e = I("hidw", [F, 2 * F]); hidb_e = I("hidb", [F, 2])
    hid2wa_e = I("hid2wa", [F, F]); hid2wb_e = I("hid2wb", [F, F])
    hid2b_e = I("hid2b", [F, 1])
    finw_e = I("finw", [F, 2]); finb_e = I("finb", [2, 1])
    out_e = nc.dram_tensor("out", [2, 1], f32, kind="ExternalOutput")

    hs_shard = nc.dram_tensor("hs_shard", [128, NB * F], bf16)
    hs_full = nc.dram_tensor("hs_full", [ROWS, F], bf16, addr_space="Shared")
    p_dram = nc.dram_tensor("p_dram", [1, F], f32)
    p_shared = nc.dram_tensor("p_shared", [1, F], f32, addr_space="Shared")
    groups_all = list(range(N_CORES))

    with tile.TileContext(nc) as tc:
        with (
            tc.tile_pool(name="const", bufs=1) as cpool,
            tc.tile_pool(name="xt", bufs=3) as xtpool,
            tc.tile_pool(name="hself", bufs=1) as hpool,
            tc.tile_pool(name="work", bufs=8) as wpool,
            tc.tile_pool(name="gat", bufs=8) as gpool,
            tc.tile_pool(name="sstile", bufs=4) as spool,
            tc.tile_pool(name="ps", bufs=2, space="PSUM") as pspool,
            tc.tile_pool(name="cellps", bufs=1, space="PSUM") as cpspool,
            tc.tile_pool(name="psp", bufs=1, space="PSUM") as psppool,
        ):
            # ---- constants
            w1_sb = cpool.tile([F, F], f32); nc.sync.dma_start(w1_sb[:], w1_e[:])
            b1_sb = cpool.tile([1, F], bf16); nc.sync.dma_start(b1_sb[:], b1_e[:])
            sq_sb = cpool.tile([1, SHARD], bf16); nc.sync.dma_start(sq_sb[:], sq_e[:])
            dinv_sb = cpool.tile([BLK, NB], f32); nc.sync.dma_start(dinv_sb[:], dinv_e[:])
            u2_sb = cpool.tile([BLK, NB], f32); nc.sync.dma_start(u2_sb[:], u2_e[:])
            iota_sb = cpool.tile([128, BLK], bf16); nc.sync.dma_start(iota_sb[:], iota_e[:])
            ident_sb = cpool.tile([128, 128], bf16); nc.sync.dma_start(ident_sb[:], ident_e[:])
            # all chunks' dst-offset columns, preloaded in one DMA
            dsta_sb = cpool.tile([128, NCHUNK * (CH_FULL // 128)], f32)
            nc.sync.dma_start(dsta_sb[:], dst_e[:])

            # ---- phase A: Hs = dinv * (X @ W1), bf16, kept in SBUF
            hs_self = hpool.tile([128, NB, F], bf16)
            for i in range(14):
                xts = xtpool.tile([F, 7 * BLK], f32, tag="xts")
                nc.sync.dma_start(xts[:], xt_e[:, i * 7 * BLK : (i + 1) * 7 * BLK])
                for j in range(7):
                    b = i * 7 + j
                    psA = pspool.tile([BLK, F], f32, tag="ps")
                    nc.tensor.matmul(psA[:], xts[:, j * BLK : (j + 1) * BLK],
                                     w1_sb[:], start=True, stop=True)
                    nc.scalar.activation(
                        hs_self[:, b, :], psA[:], mybir.ActivationFunctionType.Copy,
                        scale=dinv_sb[:, b : b + 1],
                    )
            # store shard to DRAM for the AllGather (identity layout, one DMA)
            nc.sync.dma_start(hs_shard[:], hs_self[:])

            # ---- AllGather the node table
            nc.gpsimd.collective_compute(
                "AllGather", mybir.AluOpType.bypass,
                replica_groups=[groups_all],
                ins=[hs_shard[:]], outs=[hs_full[:]],
            )

            # ---- message passing: 7 block-groups x 8 segment chunks
            # each block's 16 matmul groups accumulate purely in PSUM
            ps_p = psppool.tile([1, F], f32)
            for bg in range(CPP):
                blo = bg * 16
                ncell = min(16, NB - blo)
                chn = ncell * SEG
                psbs = {}
                for t in range((ncell + 3) // 4):
                    pst_new = cpspool.tile([BLK, 4 * F], f32, tag=f"cps{t}")
                    for q in range(min(4, ncell - t * 4)):
                        psbs[blo + t * 4 + q] = pst_new[:, q * F : (q + 1) * F]
                for k in range(NSEG):
                    ci = bg * NSEG + k
                    base = _win_base(k)
                    idxt = wpool.tile([128, CH_FULL // 16], i16, tag="idxt")
                    nc.sync.dma_start(idxt[:, : chn // 16], idx_e[ci, :, : chn // 16])
                    G = gpool.tile([128, CH_FULL // 128, F], bf16, tag="G")
                    nc.gpsimd.dma_gather(
                        out_ap=G[:, : chn // 128, :],
                        in_ap=hs_full[base : base + WIN, :],
                        idxs_ap=idxt[:, : chn // 16],
                        num_idxs=chn, num_idxs_reg=chn, elem_size=F,
                        single_packet=False, queue_num=(ci % NQUEUES),
                    )
                    for ci2 in range(ncell):
                        b = blo + ci2
                        psb = psbs[b]
                        if k == 0:
                            # bias (sqrt(deg) (x) b1) + self-loop rows
                            nc.tensor.matmul(
                                psb[:], sq_sb[:, b * BLK : (b + 1) * BLK],
                                b1_sb[:], start=True, stop=False,
                                skip_group_check=True,
                            )
                            nc.tensor.matmul(
                                psb[:], ident_sb[:], hs_self[:, b, :],
                                start=False, stop=False, skip_group_check=True,
                            )
                        for g2 in range(2):
                            g = ci2 * 2 + g2
                            S = spool.tile([128, BLK], bf16, tag="S")
                            nc.vector.tensor_scalar(
                                S[:], iota_sb[:],
                                dsta_sb[:, ci * (CH_FULL // 128) + g :
                                        ci * (CH_FULL // 128) + g + 1], None,
                                op0=mybir.AluOpType.is_equal,
                            )
                            nc.tensor.matmul(
                                psb[:], S[:], G[:, g, :],
                                start=False, stop=(k == NSEG - 1 and g2 == 1),
                                skip_group_check=True,
                            )
                    if k == NSEG - 1:
                        for ci2 in range(ncell):
                            b = blo + ci2
                            h1b = spool.tile([BLK, F], f32, tag="h1b")
                            nc.scalar.activation(
                                h1b[:], psbs[b][:],
                                mybir.ActivationFunctionType.Relu,
                                scale=dinv_sb[:, b : b + 1],
                            )
                            nc.tensor.matmul(
                                ps_p[:], u2_sb[:, b : b + 1], h1b[:],
                                start=(b == 0), stop=(b == NB - 1),
                                skip_group_check=True,
                            )

            # ---- p AllReduce
            p_sb = cpool.tile([1, F], f32)
            nc.vector.tensor_copy(p_sb[:], ps_p[:])
            nc.sync.dma_start(p_dram[:], p_sb[:])
            nc.gpsimd.collective_compute(
                "AllReduce", mybir.AluOpType.add,
                replica_groups=[groups_all],
                ins=[p_dram[:]], outs=[p_shared[:]],
            )
            p_row = cpool.tile([1, F], f32)
            nc.sync.dma_start(p_row[:], p_shared[:])
            id1 = cpool.tile([1, 1], f32)
            nc.vector.memset(id1[:], 1.0)
            psT = pspool.tile([F, 1], f32, tag="ps")
            nc.tensor.transpose(psT[:], p_row[:], id1[:])
            p_col = cpool.tile([F, 1], f32)
            nc.vector.tensor_copy(p_col[:], psT[:])

            # ---- replicated tail MLP
            tl = cpool
            def ld(e, shape, dt=f32):
                t = tl.tile(shape, dt, tag=f"c_{e.name}")
                nc.sync.dma_start(t[:], e[:])
                return t
            rates_sb = ld(rates_e, [16, 1]); encw1_sb = ld(encw1_e, [16, 8])
            encb1_sb = ld(encb1_e, [8, 1]); encw2_sb = ld(encw2_e, [8, F])
            encb2_sb = ld(encb2_e, [F, 1])
            w2a_sb = ld(w2a_e, [F, F]); w2b_sb = ld(w2b_e, [F, F])
            b2_sb = ld(b2_e, [F, 1]); s1_sb = ld(s1_e, [F, 1]); s2_sb = ld(s2_e, [F, 1])
            w3_sb = ld(w3_e, [F, F]); b3_sb = ld(b3_e, [F, 1])
            hidw_sb = ld(hidw_e, [F, 2 * F]); hidb_sb = ld(hidb_e, [F, 2])
            hid2wa_sb = ld(hid2wa_e, [F, F]); hid2wb_sb = ld(hid2wb_e, [F, F])
            hid2b_sb = ld(hid2b_e, [F, 1])
            finw_sb = ld(finw_e, [F, 2]); finb_sb = ld(finb_e, [2, 1])

            pst = pspool.tile([F, 2], f32, tag="ps")
            nc.tensor.matmul(pst[:8, 0:1], encw1_sb[:], rates_sb[:], start=True, stop=True)
            r1 = tl.tile([8, 1], f32)
            nc.scalar.activation(r1[:], pst[:8, 0:1],
                                 mybir.ActivationFunctionType.Relu, bias=encb1_sb[:])
            nc.tensor.matmul(pst[:, 1:2], encw2_sb[:], r1[:], start=True, stop=True)
            r2 = tl.tile([F, 1], f32)
            nc.vector.tensor_add(r2[:], pst[:, 1:2], encb2_sb[:])
            mr = tl.tile([F, 1], f32)
            nc.vector.tensor_mul(mr[:], r2[:], s2_sb[:])
            pst2 = pspool.tile([F, 1], f32, tag="ps")
            nc.tensor.matmul(pst2[:], w2a_sb[:], p_col[:], start=True, stop=False)
            nc.tensor.matmul(pst2[:], w2b_sb[:], mr[:], start=False, stop=True)
            sb2 = tl.tile([F, 1], f32)
            nc.vector.tensor_mul(sb2[:], b2_sb[:], s1_sb[:])
            qv = tl.tile([F, 1], f32)
            nc.vector.tensor_add(qv[:], pst2[:], sb2[:])
            nc.vector.tensor_scalar_mul(qv[:], qv[:], 1.0 / N)
            pst3 = pspool.tile([F, 1], f32, tag="ps")
            nc.tensor.matmul(pst3[:], w3_sb[:], qv[:], start=True, stop=True)
            m3 = tl.tile([F, 1], f32)
            nc.vector.tensor_add(m3[:], pst3[:], b3_sb[:])
            g1a = tl.tile([F, 1], f32); g1b = tl.tile([F, 1], f32)
            nc.tensor.matmul(pst[:, 0:1], hidw_sb[:, :F], m3[:], start=True, stop=True)
            nc.scalar.activation(g1a[:], pst[:, 0:1],
                                 mybir.ActivationFunctionType.Relu, bias=hidb_sb[:, 0:1])
            nc.tensor.matmul(pst[:, 1:2], hidw_sb[:, F:], m3[:], start=True, stop=True)
            nc.scalar.activation(g1b[:], pst[:, 1:2],
                                 mybir.ActivationFunctionType.Relu, bias=hidb_sb[:, 1:2])
            pst4 = pspool.tile([F, 1], f32, tag="ps")
            nc.tensor.matmul(pst4[:], hid2wa_sb[:], g1a[:], start=True, stop=False)
            nc.tensor.matmul(pst4[:], hid2wb_sb[:], g1b[:], start=False, stop=True)
            g2 = tl.tile([F, 1], f32)
            nc.scalar.activation(g2[:], pst4[:],
                                 mybir.ActivationFunctionType.Relu, bias=hid2b_sb[:])
            pst5 = pspool.tile([2, 1], f32, tag="ps")
            nc.tensor.matmul(pst5[:], finw_sb[:], g2[:], start=True, stop=True)
            outv = tl.tile([2, 1], f32)
            nc.vector.tensor_add(outv[:], pst5[:], finb_sb[:])
            nc.sync.dma_start(out_e[:], outv[:])

    nc.compile()
    return nc


_CACHE = {}
LAST_RESULTS = None


def kernel(**inputs):
    graph = np.asarray(inputs["graph"], np.float32)
    edge_index = np.asarray(inputs["edge_index"], np.int64)
    rates = np.asarray(inputs["rates"], np.float32)
    params = {k: np.asarray(v) for k, v in inputs.items()
              if k not in ("graph", "edge_index", "rates")}
    in_maps = _preprocess(graph, edge_index, rates, params)
    if "nc" not in _CACHE:
        _CACHE["nc"] = _build()
    nc = _CACHE["nc"]
    import os
    trace = bool(int(os.environ.get("GCN_TRACE", "0")))
    res = run_bass_kernel_spmd(nc, in_maps, list(range(N_CORES)), trace=trace)
    global LAST_RESULTS
    LAST_RESULTS = res
    out = np.asarray(res.results[0]["out"], np.float32).reshape(1, 2)
    return out


# revision 25
# speedup vs baseline: 2.9053x; 1.7911x over previous
"""GCN (3-layer + MLP head) on 8 Trainium2 NeuronCores.

Strategy (graph-parallel, dst-sharded, SWDGE-gather message passing):
  - Host renumbers nodes (LPT bin-packing on in-degree) so every 128-node
    dst block has total in-degree <= 2048; nodes shard 8 ways by new id.
  - Phase A (per core): Hs = dinv * (X_shard @ W1) as bf16 rows, kept in
    SBUF (self-loop term) and AllGathered -> full node table in HBM.
  - MP phase: each dst block's edges are sorted by src table row and split
    into 8 segments of exactly 256 edges (last = remainder); segment k of
    all blocks lies in a fixed ~32k-row window, so int16 gather indices
    cover it with zero per-cell padding.  Chunks of 16 blocks' k-segments
    (4096 idxs) feed dma_gather; per 128-edge group a one-hot S matrix
    (DVE is_equal vs iota) scatter-sums messages into the dst block via a
    PE matmul; segments accumulate into an SBUF f32 accum.
  - Self loops never enter the gather: an identity-matrix matmul adds the
    core's own Hs rows into each block's psum at segment 0.
  - GCN layers 2+3 feed only a global mean, so they collapse to weighted
    node sums (host-precomputed u2, S1, S2); p = sum_d u2[d]*relu(h1[d])
    is a PE reduction, AllReduced across cores; tiny MLP head replicated.
"""
import heapq
import numpy as np
import ml_dtypes

import concourse.bass as bass
import concourse.tile as tile
from concourse import bacc, mybir
from concourse.bass_utils import run_bass_kernel_spmd

N_CORES = 8
N = 100000
F = 128            # feature dim
BLK = 128          # dst-block size (psum partition dim)
NB = 98            # blocks per core
SHARD = NB * BLK   # 12544 rows per core (incl. pad rows)
ROWS = N_CORES * SHARD  # 100352 table rows
NSEG = 8           # segments per block
SEG = 256          # edges per segment (2 matmul groups)
GPB = NSEG * 2     # groups per block = 16
CPP = 7            # chunks per pass: 6 full (16 blocks) + 1 tail (2 blocks)
NCHUNK = NSEG * CPP
CH_FULL = 16 * SEG  # 4096 idxs
CH_TAIL = 2 * SEG   # 512 idxs
WIN = 32768
NQUEUES = 4
PAD_DST = 130.0

BF16 = ml_dtypes.bfloat16


def _win_base(k):
    return max(0, min(k * SHARD - 3584, ROWS - WIN))


# ----------------------------------------------------------------------------
# host preprocessing: renumber, normalize, sort, segment
# ----------------------------------------------------------------------------
def _preprocess(graph, edge_index, rates, params):
    src = np.asarray(edge_index[0], np.int64)
    dst = np.asarray(edge_index[1], np.int64)
    E = src.shape[0]

    # normalization scalars (f64, original ids; order-independent)
    deg = np.bincount(dst, minlength=N).astype(np.float64) + 1.0
    dinv = deg ** -0.5
    sq = deg ** 0.5
    u1 = dinv * (np.bincount(src, weights=dinv[dst], minlength=N) + dinv)
    y = u1 * dinv
    u2 = dinv * (np.bincount(src, weights=y[dst], minlength=N) + y)
    S1 = float(u1.sum())
    S2 = float(u2.sum())

    # LPT renumbering: balance per-block in-degree (target <= 2048)
    indeg = np.bincount(dst, minlength=N)
    nbins = N_CORES * NB
    order = np.argsort(-indeg, kind="stable")
    heap = [(0, 0, b) for b in range(nbins)]
    heapq.heapify(heap)
    binof = np.empty(N, np.int32)
    posof = np.empty(N, np.int32)
    for v in order:
        load, cnt, b = heapq.heappop(heap)
        binof[v] = b
        posof[v] = cnt
        cnt += 1
        load += int(indeg[v])
        if cnt < BLK:
            heapq.heappush(heap, (load, cnt, b))
    newid = binof.astype(np.int64) * BLK + posof
    # table row order is partition-major per core: row = c*SHARD + p*NB + b,
    # so the SBUF hs tile [128, NB, F] stores to DRAM as a single identity DMA
    c_ = newid // SHARD
    b_ = (newid % SHARD) // BLK
    p_ = newid % BLK
    trow = c_ * SHARD + p_ * NB + b_

    ns, nd = trow[src], newid[dst]
    cell = nd // BLK                      # global cell id = core*NB + block
    o = np.lexsort((ns, cell))
    cell_s, ns_s, off_s = cell[o], ns[o], (nd[o] % BLK).astype(np.float64)
    ccnt = np.bincount(cell_s, minlength=nbins)
    assert ccnt.max() <= NSEG * SEG, f"cell overflow {ccnt.max()}"
    cstart = np.zeros(nbins + 1, np.int64)
    cstart[1:] = np.cumsum(ccnt)

    # per-core edge tables in schedule order (block-group-major: ci = bg*8+k)
    idx16 = np.zeros((N_CORES, NCHUNK, 128, CH_FULL // 16), np.int16)
    dstid = np.full((N_CORES, NCHUNK, 128, CH_FULL // 128), PAD_DST, BF16)
    iv = np.zeros(CH_FULL, np.int64)
    ov = np.zeros(CH_FULL, np.float64)
    for c in range(N_CORES):
        for bg in range(CPP):
            blo, bhi = bg * 16, min(bg * 16 + 16, NB)
            ncell = bhi - blo
            chn = ncell * SEG
            for k in range(NSEG):
                base = _win_base(k)
                iv[:chn] = 0
                ov[:chn] = PAD_DST
                for ci2 in range(ncell):
                    g_cell = c * NB + blo + ci2
                    s0, T = cstart[g_cell], ccnt[g_cell]
                    a, b2 = k * SEG, min((k + 1) * SEG, T)
                    if a >= b2:
                        continue
                    take = b2 - a
                    rel = ns_s[s0 + a : s0 + b2] - base
                    assert rel.min() >= 0 and rel.max() < WIN, (c, k, bg, ci2)
                    p0 = ci2 * SEG
                    iv[p0 : p0 + take] = rel
                    ov[p0 : p0 + take] = off_s[s0 + a : s0 + b2]
                ci = bg * NSEG + k
                e = np.arange(chn)
                tmp = np.zeros((16, CH_FULL // 16), np.int16)
                tmp[e % 16, e // 16] = iv[:chn]
                idx16[c, ci] = np.tile(tmp, (8, 1))
                dstid[c, ci][e % 128, e // 128] = ov[:chn].astype(BF16)

    # phase A inputs (new ordering, padded)
    X = np.asarray(graph, np.float32)
    inv = np.full(ROWS, -1, np.int64)
    inv[newid] = np.arange(N)
    xt = np.zeros((N_CORES, F, SHARD), np.float32)
    dinv_pm = np.zeros((N_CORES, BLK, NB), np.float32)
    u2_pm = np.zeros((N_CORES, BLK, NB), np.float32)
    sqdeg = np.zeros((N_CORES, 1, SHARD), np.float32)
    dv = np.zeros(ROWS)
    uv = np.zeros(ROWS)
    sv = np.zeros(ROWS)
    real = inv >= 0
    dv[real] = dinv[inv[real]]
    uv[real] = u2[inv[real]]
    sv[real] = sq[inv[real]]
    for c in range(N_CORES):
        rows = inv[c * SHARD : (c + 1) * SHARD]
        m = rows >= 0
        xt[c][:, m] = X[rows[m]].T
        dinv_pm[c] = dv[c * SHARD : (c + 1) * SHARD].reshape(NB, BLK).T
        u2_pm[c] = uv[c * SHARD : (c + 1) * SHARD].reshape(NB, BLK).T
        sqdeg[c, 0] = sv[c * SHARD : (c + 1) * SHARD]

    p = params
    col = lambda v: np.asarray(v, np.float32).reshape(-1, 1)
    iota = np.tile(np.arange(BLK, dtype=BF16)[None, :], (128, 1))
    ident = np.eye(128, dtype=BF16)
    common = dict(
        w1=np.asarray(p["conv1_W"], np.float32),
        b1row=np.asarray(p["conv1_b"], BF16).reshape(1, F),
        iota=iota,
        ident=ident,
        rates_col=col(rates),
        encw1=np.asarray(p["enc_W1"], np.float32),
        encb1=col(p["enc_b1"]),
        encw2=np.asarray(p["enc_W2"], np.float32),
        encb2=col(p["enc_b2"]),
        w2a=np.asarray(p["conv2_W"], np.float32)[:F],
        w2b=np.asarray(p["conv2_W"], np.float32)[F:],
        b2col=col(p["conv2_b"]),
        s1col=np.full((F, 1), S1, np.float32),
        s2col=np.full((F, 1), S2, np.float32),
        w3=np.asarray(p["conv3_W"], np.float32),
        b3col=col(p["conv3_b"]),
        hidw=np.asarray(p["hid_W"], np.float32),
        hidb=np.asarray(p["hid_b"], np.float32).reshape(2, F).T,
        hid2wa=np.asarray(p["hid2_W"], np.float32)[:F],
        hid2wb=np.asarray(p["hid2_W"], np.float32)[F:],
        hid2b=col(p["hid2_b"]),
        finw=np.asarray(p["fin_W"], np.float32),
        finb=col(p["fin_b"]),
    )
    in_maps = []
    for c in range(N_CORES):
        m = dict(common)
        m.update(
            xt=xt[c], sqdeg=sqdeg[c].astype(BF16), dinv=dinv_pm[c],
            u2c=u2_pm[c], idx16=idx16[c],
            dstid=np.ascontiguousarray(dstid[c].transpose(1, 0, 2)),
        )
        in_maps.append(m)
    return in_maps


# ----------------------------------------------------------------------------
# device program
# ----------------------------------------------------------------------------
def _build():
    f32, bf16, i16 = mybir.dt.float32, mybir.dt.bfloat16, mybir.dt.int16

    nc = bacc.Bacc("TRN2", target_bir_lowering=False, debug=False,
                   num_devices=N_CORES, num_swdge_queues=NQUEUES)
    I = lambda name, shape, dt=f32: nc.dram_tensor(name, shape, dt, kind="ExternalInput")
    xt_e = I("xt", [F, SHARD])
    w1_e = I("w1", [F, F]); b1_e = I("b1row", [1, F], bf16)
    sq_e = I("sqdeg", [1, SHARD], bf16)
    dinv_e = I("dinv", [BLK, NB]); u2_e = I("u2c", [BLK, NB])
    idx_e = I("idx16", [NCHUNK, 128, CH_FULL // 16], i16)
    dst_e = I("dstid", [128, NCHUNK, CH_FULL // 128], bf16)
    iota_e = I("iota", [128, BLK], bf16)
    ident_e = I("ident", [128, 128], bf16)
    rates_e = I("rates_col", [16, 1])
    encw1_e = I("encw1", [16, 8]); encb1_e = I("encb1", [8, 1])
    encw2_e = I("encw2", [8, F]); encb2_e = I("encb2", [F, 1])
    w2a_e = I("w2a", [F, F]); w2b_e = I("w2b", [F, F]); b2_e = I("b2col", [F, 1])
    s1_e = I("s1col", [F, 1]); s2_e = I("s2col", [F, 1])
    w3_e = I("w3", [F, F]); b3_e = I("b3col", [F, 1])
    hidw_e = I("hidw", [F, 2 * F]); hidb_e = I("hidb", [F, 2])
    hid2wa_e = I("hid2wa", [F, F]); hid2wb_e = I("hid2wb", [F, F])
    hid2b_e = I("hid2b", [F, 1])
    finw_e = I("finw", [F, 2]); finb_e = I("finb", [2, 1])
    out_e = nc.dram_tensor("out", [2, 1], f32, kind="ExternalOutput")

    hs_shard = nc.dram_tensor("hs_shard", [128, NB * F], bf16)
    hs_full = nc.dram_tensor("hs_full", [ROWS, F], bf16, addr_space="Shared")
    p_dram = nc.dram_tensor("p_dram", [1, F], f32)
    p_shared = nc.dram_tensor("p_shared", [1, F], f32, addr_space="Shared")
    groups_all = list(range(N_CORES))

    with tile.TileContext(nc) as tc:
        with (
            tc.tile_pool(name="const", bufs=1) as cpool,
            tc.tile_pool(name="xt", bufs=3) as xtpool,
            tc.tile_pool(name="hself", bufs=1) as hpool,
            tc.tile_pool(name="work", bufs=8) as wpool,
            tc.tile_pool(name="gat", bufs=8) as gpool,
            tc.tile_pool(name="sstile", bufs=4) as spool,
            tc.tile_pool(name="ps", bufs=2, space="PSUM") as pspool,
            tc.tile_pool(name="cellps", bufs=1, space="PSUM") as cpspool,
            tc.tile_pool(name="psp", bufs=1, space="PSUM") as psppool,
        ):
            # ---- constants
            w1_sb = cpool.tile([F, F], f32); nc.sync.dma_start(w1_sb[:], w1_e[:])
            b1_sb = cpool.tile([1, F], bf16); nc.sync.dma_start(b1_sb[:], b1_e[:])
            sq_sb = cpool.tile([1, SHARD], bf16); nc.sync.dma_start(sq_sb[:], sq_e[:])
            dinv_sb = cpool.tile([BLK, NB], f32); nc.sync.dma_start(dinv_sb[:], dinv_e[:])
            u2_sb = cpool.tile([BLK, NB], f32); nc.sync.dma_start(u2_sb[:], u2_e[:])
            iota_sb = cpool.tile([128, BLK], bf16); nc.sync.dma_start(iota_sb[:], iota_e[:])
            ident_sb = cpool.tile([128, 128], bf16); nc.sync.dma_start(ident_sb[:], ident_e[:])
            # all chunks' dst-offset columns, preloaded in one DMA
            dsta_sb = cpool.tile([128, NCHUNK * (CH_FULL // 128)], bf16)
            nc.scalar.dma_start(dsta_sb[:], dst_e[:])

            # ---- phase A: Hs = dinv * (X @ W1), bf16, kept in SBUF
            hs_self = hpool.tile([128, NB, F], bf16)
            for i in range(14):
                xts = xtpool.tile([F, 7 * BLK], f32, tag="xts")
                nc.sync.dma_start(xts[:], xt_e[:, i * 7 * BLK : (i + 1) * 7 * BLK])
                for j in range(7):
                    b = i * 7 + j
                    psA = pspool.tile([BLK, F], f32, tag="ps")
                    nc.tensor.matmul(psA[:], xts[:, j * BLK : (j + 1) * BLK],
                                     w1_sb[:], start=True, stop=True)
                    nc.scalar.activation(
                        hs_self[:, b, :], psA[:], mybir.ActivationFunctionType.Copy,
                        scale=dinv_sb[:, b : b + 1],
                    )
            # store shard to DRAM for the AllGather (identity layout, one DMA)
            nc.sync.dma_start(hs_shard[:], hs_self[:])

            # ---- AllGather the node table
            nc.gpsimd.collective_compute(
                "AllGather", mybir.AluOpType.bypass,
                replica_groups=[groups_all],
                ins=[hs_shard[:]], outs=[hs_full[:]],
            )

            # ---- message passing: 7 block-groups x 8 segment chunks
            # each block's 16 matmul groups accumulate purely in PSUM
            ps_p = psppool.tile([1, F], f32)
            for bg in range(CPP):
                blo = bg * 16
                ncell = min(16, NB - blo)
                chn = ncell * SEG
                psbs = {}
                for t in range((ncell + 3) // 4):
                    pst_new = cpspool.tile([BLK, 4 * F], f32, tag=f"cps{t}")
                    for q in range(min(4, ncell - t * 4)):
                        psbs[blo + t * 4 + q] = pst_new[:, q * F : (q + 1) * F]
                for k in range(NSEG):
                    ci = bg * NSEG + k
                    base = _win_base(k)
                    idxt = wpool.tile([128, CH_FULL // 16], i16, tag="idxt")
                    nc.scalar.dma_start(idxt[:, : chn // 16], idx_e[ci, :, : chn // 16])
                    G = gpool.tile([128, CH_FULL // 128, F], bf16, tag="G")
                    nc.gpsimd.dma_gather(
                        out_ap=G[:, : chn // 128, :],
                        in_ap=hs_full[base : base + WIN, :],
                        idxs_ap=idxt[:, : chn // 16],
                        num_idxs=chn, num_idxs_reg=chn, elem_size=F,
                        single_packet=False, queue_num=(ci % NQUEUES),
                    )
                    Sts = []
                    for sg in range((chn // 128 + 3) // 4):
                        S4 = spool.tile([128, 4, BLK], bf16, tag=f"S{sg % 2}")
                        iap = iota_sb[:]
                        in0 = bass.AP(iap.tensor, iap.offset,
                                      [iap.ap[0], [0, 4], iap.ap[1]])
                        c0 = ci * (CH_FULL // 128) + sg * 4
                        dap = dsta_sb[:, c0 : c0 + 4]
                        in1 = bass.AP(dap.tensor, dap.offset,
                                      [dap.ap[0], dap.ap[1], [0, 128]])
                        nc.vector.tensor_tensor(S4[:], in0, in1,
                                                mybir.AluOpType.is_equal)
                        Sts.append(S4)
                    for ci2 in range(ncell):
                        b = blo + ci2
                        psb = psbs[b]
                        if k == 0:
                            # bias (sqrt(deg) (x) b1) + self-loop rows
                            nc.tensor.matmul(
                                psb[:], sq_sb[:, b * BLK : (b + 1) * BLK],
                                b1_sb[:], start=True, stop=False,
                                skip_group_check=True,
                            )
                            nc.tensor.matmul(
                                psb[:], ident_sb[:], hs_self[:, b, :],
                                start=False, stop=False, skip_group_check=True,
                            )
                        for g2 in range(2):
                            g = ci2 * 2 + g2
                            nc.tensor.matmul(
                                psb[:], Sts[g // 4][:, g % 4, :], G[:, g, :],
                                start=False, stop=(k == NSEG - 1 and g2 == 1),
                                skip_group_check=True,
                            )
                    if k == NSEG - 1:
                        for ci2 in range(ncell):
                            b = blo + ci2
                            h1b = spool.tile([BLK, F], f32, tag="h1b")
                            nc.scalar.activation(
                                h1b[:], psbs[b][:],
                                mybir.ActivationFunctionType.Relu,
                                scale=dinv_sb[:, b : b + 1],
                            )
                            nc.tensor.matmul(
                                ps_p[:], u2_sb[:, b : b + 1], h1b[:],
                                start=(b == 0), stop=(b == NB - 1),
                                skip_group_check=True,
                            )

            # ---- p AllReduce
            p_sb = cpool.tile([1, F], f32)
            nc.vector.tensor_copy(p_sb[:], ps_p[:])
            nc.sync.dma_start(p_dram[:], p_sb[:])
            nc.gpsimd.collective_compute(
                "AllReduce", mybir.AluOpType.add,
                replica_groups=[groups_all],
                ins=[p_dram[:]], outs=[p_shared[:]],
            )
            p_row = cpool.tile([1, F], f32)
            nc.sync.dma_start(p_row[:], p_shared[:])
            id1 = cpool.tile([1, 1], f32)
            nc.vector.memset(id1[:], 1.0)
            psT = pspool.tile([F, 1], f32, tag="ps")
            nc.tensor.transpose(psT[:], p_row[:], id1[:])
            p_col = cpool.tile([F, 1], f32)
            nc.vector.tensor_copy(p_col[:], psT[:])

            # ---- replicated tail MLP
            tl = cpool
            def ld(e, shape, dt=f32):
                t = tl.tile(shape, dt, tag=f"c_{e.name}")
                nc.sync.dma_start(t[:], e[:])
                return t
            rates_sb = ld(rates_e, [16, 1]); encw1_sb = ld(encw1_e, [16, 8])
            encb1_sb = ld(encb1_e, [8, 1]); encw2_sb = ld(encw2_e, [8, F])
            encb2_sb = ld(encb2_e, [F, 1])
            w2a_sb = ld(w2a_e, [F, F]); w2b_sb = ld(w2b_e, [F, F])
            b2_sb = ld(b2_e, [F, 1]); s1_sb = ld(s1_e, [F, 1]); s2_sb = ld(s2_e, [F, 1])
            w3_sb = ld(w3_e, [F, F]); b3_sb = ld(b3_e, [F, 1])
            hidw_sb = ld(hidw_e, [F, 2 * F]); hidb_sb = ld(hidb_e, [F, 2])
            hid2wa_sb = ld(hid2wa_e, [F, F]); hid2wb_sb = ld(hid2wb_e, [F, F])
            hid2b_sb = ld(hid2b_e, [F, 1])
            finw_sb = ld(finw_e, [F, 2]); finb_sb = ld(finb_e, [2, 1])

            pst = pspool.tile([F, 2], f32, tag="ps")
            nc.tensor.matmul(pst[:8, 0:1], encw1_sb[:], rates_sb[:], start=True, stop=True)
            r1 = tl.tile([8, 1], f32)
            nc.scalar.activation(r1[:], pst[:8, 0:1],
                                 mybir.ActivationFunctionType.Relu, bias=encb1_sb[:])
            nc.tensor.matmul(pst[:, 1:2], encw2_sb[:], r1[:], start=True, stop=True)
            r2 = tl.tile([F, 1], f32)
            nc.vector.tensor_add(r2[:], pst[:, 1:2], encb2_sb[:])
            mr = tl.tile([F, 1], f32)
            nc.vector.tensor_mul(mr[:], r2[:], s2_sb[:])
            pst2 = pspool.tile([F, 1], f32, tag="ps")
            nc.tensor.matmul(pst2[:], w2a_sb[:], p_col[:], start=True, stop=False)
            nc.tensor.matmul(pst2[:], w2b_sb[:], mr[:], start=False, stop=True)
            sb2 = tl.tile([F, 1], f32)
            nc.vector.tensor_mul(sb2[:], b2_sb[:], s1_sb[:])
            qv = tl.tile([F, 1], f32)
            nc.vector.tensor_add(qv[:], pst2[:], sb2[:])
            nc.vector.tensor_scalar_mul(qv[:], qv[:], 1.0 / N)
            pst3 = pspool.tile([F, 1], f32, tag="ps")
            nc.tensor.matmul(pst3[:], w3_sb[:], qv[:], start=True, stop=True)
            m3 = tl.tile([F, 1], f32)
            nc.vector.tensor_add(m3[:], pst3[:], b3_sb[:])
            g1a = tl.tile([F, 1], f32); g1b = tl.tile([F, 1], f32)
            nc.tensor.matmul(pst[:, 0:1], hidw_sb[:, :F], m3[:], start=True, stop=True)
            nc.scalar.activation(g1a[:], pst[:, 0:1],
                                 mybir.ActivationFunctionType.Relu, bias=hidb_sb[:, 0:1])
            nc.tensor.matmul(pst[:, 1:2], hidw_sb[:, F:], m3[:], start=True, stop=True)
            nc.scalar.activation(g1b[:], pst[:, 1:2],
                                 mybir.ActivationFunctionType.Relu, bias=hidb_sb[:, 1:2])
            pst4 = pspool.tile([F, 1], f32, tag="ps")
            nc.tensor.matmul(pst4[:], hid2wa_sb[:], g1a[:], start=True, stop=False)
            nc.tensor.matmul(pst4[:], hid2wb_sb[:], g1b[:], start=False, stop=True)
            g2 = tl.tile([F, 1], f32)
            nc.scalar.activation(g2[:], pst4[:],
                                 mybir.ActivationFunctionType.Relu, bias=hid2b_sb[:])
            pst5 = pspool.tile([2, 1], f32, tag="ps")
            nc.tensor.matmul(pst5[:], finw_sb[:], g2[:], start=True, stop=True)
            outv = tl.tile([2, 1], f32)
            nc.vector.tensor_add(outv[:], pst5[:], finb_sb[:])
            nc.sync.dma_start(out_e[:], outv[:])

    nc.compile()
    return nc


_CACHE = {}
LAST_RESULTS = None


def kernel(**inputs):
    graph = np.asarray(inputs["graph"], np.float32)
    edge_index = np.asarray(inputs["edge_index"], np.int64)
    rates = np.asarray(inputs["rates"], np.float32)
    params = {k: np.asarray(v) for k, v in inputs.items()
              if k not in ("graph", "edge_index", "rates")}
    in_maps = _preprocess(graph, edge_index, rates, params)
    if "nc" not in _CACHE:
        _CACHE["nc"] = _build()
    nc = _CACHE["nc"]
    import os
    trace = bool(int(os.environ.get("GCN_TRACE", "0")))
    res = run_bass_kernel_spmd(nc, in_maps, list(range(N_CORES)), trace=trace)
    global LAST_RESULTS
    LAST_RESULTS = res
    out = np.asarray(res.results[0]["out"], np.float32).reshape(1, 2)
    return out


# revision 27
# speedup vs baseline: 4.1577x; 1.4311x over previous
"""GCN (3-layer + MLP head) on 8 Trainium2 NeuronCores.

Strategy (graph-parallel, dst-sharded, SWDGE-gather message passing):
  - Host renumbers nodes (LPT bin-packing on in-degree) so every 128-node
    dst block has total in-degree <= 2048; nodes shard 8 ways by new id.
  - Phase A (per core): Hs = dinv * (X_shard @ W1) as bf16 rows, kept in
    SBUF (self-loop term) and AllGathered -> full node table in HBM.
  - MP phase: each dst block's edges are sorted by src table row and split
    into 8 segments of exactly 256 edges (last = remainder); segment k of
    all blocks lies in a fixed ~32k-row window, so int16 gather indices
    cover it with zero per-cell padding.  Chunks of 16 blocks' k-segments
    (4096 idxs) feed dma_gather; per 128-edge group a one-hot S matrix
    (DVE is_equal vs iota) scatter-sums messages into the dst block via a
    PE matmul; segments accumulate into an SBUF f32 accum.
  - Self loops never enter the gather: an identity-matrix matmul adds the
    core's own Hs rows into each block's psum at segment 0.
  - GCN layers 2+3 feed only a global mean, so they collapse to weighted
    node sums (host-precomputed u2, S1, S2); p = sum_d u2[d]*relu(h1[d])
    is a PE reduction, AllReduced across cores; tiny MLP head replicated.
"""
import heapq
import numpy as np
import ml_dtypes

import concourse.bass as bass
import concourse.tile as tile
from concourse import bacc, mybir
from concourse.bass_utils import run_bass_kernel_spmd

N_CORES = 8
N = 100000
F = 128            # feature dim
BLK = 128          # dst-block size (psum partition dim)
NB = 98            # blocks per core
SHARD = NB * BLK   # 12544 rows per core (incl. pad rows)
ROWS = N_CORES * SHARD  # 100352 table rows
NSEG = 8           # segments per block
SEG = 256          # edges per segment (2 matmul groups)
GPB = NSEG * 2     # groups per block = 16
BPG = 14           # blocks per group (98 = 7*14)
CPP = 7            # block-groups
NCHUNK = NSEG * CPP
CH_FULL = BPG * SEG  # 3584 idxs
WIN = 32768
NQUEUES = 4
PAD_DST = 130.0

BF16 = ml_dtypes.bfloat16


def _win_base(k):
    return max(0, min(k * SHARD - 3584, ROWS - WIN))


# ----------------------------------------------------------------------------
# host preprocessing: renumber, normalize, sort, segment
# ----------------------------------------------------------------------------
def _preprocess(graph, edge_index, rates, params):
    src = np.asarray(edge_index[0], np.int64)
    dst = np.asarray(edge_index[1], np.int64)
    E = src.shape[0]

    # normalization scalars (f64, original ids; order-independent)
    deg = np.bincount(dst, minlength=N).astype(np.float64) + 1.0
    dinv = deg ** -0.5
    sq = deg ** 0.5
    u1 = dinv * (np.bincount(src, weights=dinv[dst], minlength=N) + dinv)
    y = u1 * dinv
    u2 = dinv * (np.bincount(src, weights=y[dst], minlength=N) + y)
    S1 = float(u1.sum())
    S2 = float(u2.sum())

    # LPT renumbering: balance per-block in-degree (target <= 2048)
    indeg = np.bincount(dst, minlength=N)
    nbins = N_CORES * NB
    order = np.argsort(-indeg, kind="stable")
    heap = [(0, 0, b) for b in range(nbins)]
    heapq.heapify(heap)
    binof = np.empty(N, np.int32)
    posof = np.empty(N, np.int32)
    for v in order:
        load, cnt, b = heapq.heappop(heap)
        binof[v] = b
        posof[v] = cnt
        cnt += 1
        load += int(indeg[v])
        if cnt < BLK:
            heapq.heappush(heap, (load, cnt, b))
    newid = binof.astype(np.int64) * BLK + posof
    # table row order is partition-major per core: row = c*SHARD + p*NB + b,
    # so the SBUF hs tile [128, NB, F] stores to DRAM as a single identity DMA
    c_ = newid // SHARD
    b_ = (newid % SHARD) // BLK
    p_ = newid % BLK
    trow = c_ * SHARD + p_ * NB + b_

    ns, nd = trow[src], newid[dst]
    cell = nd // BLK                      # global cell id = core*NB + block
    o = np.lexsort((ns, cell))
    cell_s, ns_s, off_s = cell[o], ns[o], (nd[o] % BLK).astype(np.float64)
    ccnt = np.bincount(cell_s, minlength=nbins)
    assert ccnt.max() <= NSEG * SEG, f"cell overflow {ccnt.max()}"
    cstart = np.zeros(nbins + 1, np.int64)
    cstart[1:] = np.cumsum(ccnt)

    # per-core edge tables in schedule order (block-group-major: ci = bg*8+k)
    idx16 = np.zeros((N_CORES, NCHUNK, 128, CH_FULL // 16), np.int16)
    dstid = np.full((N_CORES, NCHUNK, 128, CH_FULL // 128), PAD_DST, BF16)
    iv = np.zeros(CH_FULL, np.int64)
    ov = np.zeros(CH_FULL, np.float64)
    for c in range(N_CORES):
        for bg in range(CPP):
            blo, bhi = bg * BPG, (bg + 1) * BPG
            ncell = bhi - blo
            chn = ncell * SEG
            for k in range(NSEG):
                base = _win_base(k)
                iv[:chn] = 0
                ov[:chn] = PAD_DST
                for ci2 in range(ncell):
                    g_cell = c * NB + blo + ci2
                    s0, T = cstart[g_cell], ccnt[g_cell]
                    a, b2 = k * SEG, min((k + 1) * SEG, T)
                    if a >= b2:
                        continue
                    take = b2 - a
                    rel = ns_s[s0 + a : s0 + b2] - base
                    assert rel.min() >= 0 and rel.max() < WIN, (c, k, bg, ci2)
                    p0 = ci2 * SEG
                    iv[p0 : p0 + take] = rel
                    ov[p0 : p0 + take] = off_s[s0 + a : s0 + b2]
                ci = bg * NSEG + k
                e = np.arange(chn)
                tmp = np.zeros((16, CH_FULL // 16), np.int16)
                tmp[e % 16, e // 16] = iv[:chn]
                idx16[c, ci] = np.tile(tmp, (8, 1))
                dstid[c, ci][e % 128, e // 128] = ov[:chn].astype(BF16)

    # phase A inputs (new ordering, padded)
    X = np.asarray(graph, np.float32)
    inv = np.full(ROWS, -1, np.int64)
    inv[newid] = np.arange(N)
    xt = np.zeros((N_CORES, F, SHARD), np.float32)
    dinv_pm = np.zeros((N_CORES, BLK, NB), np.float32)
    u2_pm = np.zeros((N_CORES, BLK, NB), np.float32)
    sqdeg = np.zeros((N_CORES, 1, SHARD), np.float32)
    dv = np.zeros(ROWS)
    uv = np.zeros(ROWS)
    sv = np.zeros(ROWS)
    real = inv >= 0
    dv[real] = dinv[inv[real]]
    uv[real] = u2[inv[real]]
    sv[real] = sq[inv[real]]
    for c in range(N_CORES):
        rows = inv[c * SHARD : (c + 1) * SHARD]
        m = rows >= 0
        xt[c][:, m] = X[rows[m]].T
        dinv_pm[c] = dv[c * SHARD : (c + 1) * SHARD].reshape(NB, BLK).T
        u2_pm[c] = uv[c * SHARD : (c + 1) * SHARD].reshape(NB, BLK).T
        sqdeg[c, 0] = sv[c * SHARD : (c + 1) * SHARD]

    p = params
    col = lambda v: np.asarray(v, np.float32).reshape(-1, 1)
    iota = np.tile(np.arange(BLK, dtype=BF16)[None, :], (128, 1))
    ident = np.eye(128, dtype=BF16)
    common = dict(
        w1=np.asarray(p["conv1_W"], np.float32),
        b1row=np.asarray(p["conv1_b"], BF16).reshape(1, F),
        iota=iota,
        ident=ident,
        rates_col=col(rates),
        encw1=np.asarray(p["enc_W1"], np.float32),
        encb1=col(p["enc_b1"]),
        encw2=np.asarray(p["enc_W2"], np.float32),
        encb2=col(p["enc_b2"]),
        w2a=np.asarray(p["conv2_W"], np.float32)[:F],
        w2b=np.asarray(p["conv2_W"], np.float32)[F:],
        b2col=col(p["conv2_b"]),
        s1col=np.full((F, 1), S1, np.float32),
        s2col=np.full((F, 1), S2, np.float32),
        w3=np.asarray(p["conv3_W"], np.float32),
        b3col=col(p["conv3_b"]),
        hidw=np.asarray(p["hid_W"], np.float32),
        hidb=np.asarray(p["hid_b"], np.float32).reshape(2, F).T,
        hid2wa=np.asarray(p["hid2_W"], np.float32)[:F],
        hid2wb=np.asarray(p["hid2_W"], np.float32)[F:],
        hid2b=col(p["hid2_b"]),
        finw=np.asarray(p["fin_W"], np.float32),
        finb=col(p["fin_b"]),
    )
    in_maps = []
    for c in range(N_CORES):
        m = dict(common)
        m.update(
            xt=xt[c], sqdeg=sqdeg[c].astype(BF16), dinv=dinv_pm[c],
            u2c=u2_pm[c], idx16=idx16[c],
            dstid=np.ascontiguousarray(dstid[c].transpose(1, 0, 2)),
        )
        in_maps.append(m)
    return in_maps


# ----------------------------------------------------------------------------
# device program
# ----------------------------------------------------------------------------
def _build():
    f32, bf16, i16 = mybir.dt.float32, mybir.dt.bfloat16, mybir.dt.int16

    nc = bacc.Bacc("TRN2", target_bir_lowering=False, debug=False,
                   num_devices=N_CORES, num_swdge_queues=NQUEUES)
    I = lambda name, shape, dt=f32: nc.dram_tensor(name, shape, dt, kind="ExternalInput")
    xt_e = I("xt", [F, SHARD])
    w1_e = I("w1", [F, F]); b1_e = I("b1row", [1, F], bf16)
    sq_e = I("sqdeg", [1, SHARD], bf16)
    dinv_e = I("dinv", [BLK, NB]); u2_e = I("u2c", [BLK, NB])
    idx_e = I("idx16", [NCHUNK, 128, CH_FULL // 16], i16)
    dst_e = I("dstid", [128, NCHUNK, CH_FULL // 128], bf16)
    iota_e = I("iota", [128, BLK], bf16)
    ident_e = I("ident", [128, 128], bf16)
    rates_e = I("rates_col", [16, 1])
    encw1_e = I("encw1", [16, 8]); encb1_e = I("encb1", [8, 1])
    encw2_e = I("encw2", [8, F]); encb2_e = I("encb2", [F, 1])
    w2a_e = I("w2a", [F, F]); w2b_e = I("w2b", [F, F]); b2_e = I("b2col", [F, 1])
    s1_e = I("s1col", [F, 1]); s2_e = I("s2col", [F, 1])
    w3_e = I("w3", [F, F]); b3_e = I("b3col", [F, 1])
    hidw_e = I("hidw", [F, 2 * F]); hidb_e = I("hidb", [F, 2])
    hid2wa_e = I("hid2wa", [F, F]); hid2wb_e = I("hid2wb", [F, F])
    hid2b_e = I("hid2b", [F, 1])
    finw_e = I("finw", [F, 2]); finb_e = I("finb", [2, 1])
    out_e = nc.dram_tensor("out", [2, 1], f32, kind="ExternalOutput")

    hs_shard = nc.dram_tensor("hs_shard", [128, NB * F], bf16)
    hs_full = nc.dram_tensor("hs_full", [ROWS, F], bf16, addr_space="Shared")
    p_dram = nc.dram_tensor("p_dram", [1, F], f32)
    p_shared = nc.dram_tensor("p_shared", [1, F], f32, addr_space="Shared")
    groups_all = list(range(N_CORES))

    with tile.TileContext(nc) as tc:
        with (
            tc.tile_pool(name="const", bufs=1) as cpool,
            tc.tile_pool(name="xt", bufs=3) as xtpool,
            tc.tile_pool(name="hself", bufs=1) as hpool,
            tc.tile_pool(name="work", bufs=8) as wpool,
            tc.tile_pool(name="gat", bufs=8) as gpool,
            tc.tile_pool(name="sstile", bufs=4) as spool,
            tc.tile_pool(name="ps", bufs=2, space="PSUM") as pspool,
            tc.tile_pool(name="cellps", bufs=1, space="PSUM") as cpspool,
            tc.tile_pool(name="psp", bufs=1, space="PSUM") as psppool,
        ):
            # ---- constants
            w1_sb = cpool.tile([F, F], f32); nc.sync.dma_start(w1_sb[:], w1_e[:])
            b1_sb = cpool.tile([1, F], bf16); nc.sync.dma_start(b1_sb[:], b1_e[:])
            sq_sb = cpool.tile([1, SHARD], bf16); nc.sync.dma_start(sq_sb[:], sq_e[:])
            dinv_sb = cpool.tile([BLK, NB], f32); nc.sync.dma_start(dinv_sb[:], dinv_e[:])
            u2_sb = cpool.tile([BLK, NB], f32); nc.sync.dma_start(u2_sb[:], u2_e[:])
            iota_sb = cpool.tile([128, BLK], bf16); nc.sync.dma_start(iota_sb[:], iota_e[:])
            ident_sb = cpool.tile([128, 128], bf16); nc.sync.dma_start(ident_sb[:], ident_e[:])
            # all chunks' dst-offset columns, preloaded in one DMA
            dsta_sb = cpool.tile([128, NCHUNK * (CH_FULL // 128)], bf16)
            nc.sync.dma_start(dsta_sb[:], dst_e[:])

            # ---- phase A: Hs = dinv * (X @ W1), bf16, kept in SBUF
            hs_self = hpool.tile([128, NB, F], bf16)
            for i in range(14):
                xts = xtpool.tile([F, 7 * BLK], f32, tag="xts")
                nc.sync.dma_start(xts[:], xt_e[:, i * 7 * BLK : (i + 1) * 7 * BLK])
                for j in range(7):
                    b = i * 7 + j
                    psA = pspool.tile([BLK, F], f32, tag="ps")
                    nc.tensor.matmul(psA[:], xts[:, j * BLK : (j + 1) * BLK],
                                     w1_sb[:], start=True, stop=True)
                    nc.scalar.activation(
                        hs_self[:, b, :], psA[:], mybir.ActivationFunctionType.Copy,
                        scale=dinv_sb[:, b : b + 1],
                    )
                if i % 2 == 1:
                    # store finished 14-block stretch while phase A continues
                    lo = (i - 1) * 7 * F
                    nc.sync.dma_start(hs_shard[:, lo : lo + 14 * F],
                                      hs_self[:, (i - 1) * 7 : (i + 1) * 7, :])

            # ---- AllGather the node table
            nc.gpsimd.collective_compute(
                "AllGather", mybir.AluOpType.bypass,
                replica_groups=[groups_all],
                ins=[hs_shard[:]], outs=[hs_full[:]],
            )

            # ---- message passing: 7 block-groups x 8 segment chunks
            # each block's 16 matmul groups accumulate purely in PSUM
            ps_p = psppool.tile([1, F], f32)
            for bg in range(CPP):
                blo = bg * BPG
                ncell = BPG
                chn = ncell * SEG
                psbs = {}
                for t in range((ncell + 3) // 4):
                    pst_new = cpspool.tile([BLK, 4 * F], f32, tag=f"cps{t}")
                    for q in range(min(4, ncell - t * 4)):
                        psbs[blo + t * 4 + q] = pst_new[:, q * F : (q + 1) * F]
                for k in range(NSEG):
                    ci = bg * NSEG + k
                    base = _win_base(k)
                    idxt = wpool.tile([128, CH_FULL // 16], i16, tag="idxt")
                    nc.sync.dma_start(idxt[:, : chn // 16], idx_e[ci, :, : chn // 16])
                    G = gpool.tile([128, CH_FULL // 128, F], bf16, tag="G")
                    nc.gpsimd.dma_gather(
                        out_ap=G[:, : chn // 128, :],
                        in_ap=hs_full[base : base + WIN, :],
                        idxs_ap=idxt[:, : chn // 16],
                        num_idxs=chn, num_idxs_reg=chn, elem_size=F,
                        single_packet=False, queue_num=(ci % NQUEUES),
                    )
                    Sts = []
                    for sg in range((chn // 128 + 3) // 4):
                        S4 = spool.tile([128, 4, BLK], bf16, tag=f"S{sg % 2}")
                        iap = iota_sb[:]
                        in0 = bass.AP(iap.tensor, iap.offset,
                                      [iap.ap[0], [0, 4], iap.ap[1]])
                        c0 = ci * (CH_FULL // 128) + sg * 4
                        dap = dsta_sb[:, c0 : c0 + 4]
                        in1 = bass.AP(dap.tensor, dap.offset,
                                      [dap.ap[0], dap.ap[1], [0, 128]])
                        nc.vector.tensor_tensor(S4[:], in0, in1,
                                                mybir.AluOpType.is_equal)
                        Sts.append(S4)
                    for ci2 in range(ncell):
                        b = blo + ci2
                        psb = psbs[b]
                        if k == 0:
                            # bias (sqrt(deg) (x) b1) + self-loop rows
                            nc.tensor.matmul(
                                psb[:], sq_sb[:, b * BLK : (b + 1) * BLK],
                                b1_sb[:], start=True, stop=False,
                                skip_group_check=True,
                            )
                            nc.tensor.matmul(
                                psb[:], ident_sb[:], hs_self[:, b, :],
                                start=False, stop=False, skip_group_check=True,
                            )
                        for g2 in range(2):
                            g = ci2 * 2 + g2
                            nc.tensor.matmul(
                                psb[:], Sts[g // 4][:, g % 4, :], G[:, g, :],
                                start=False, stop=(k == NSEG - 1 and g2 == 1),
                                skip_group_check=True,
                            )
                    if k == NSEG - 1:
                        for ci2 in range(ncell):
                            b = blo + ci2
                            h1b = spool.tile([BLK, F], f32, tag="h1b")
                            nc.scalar.activation(
                                h1b[:], psbs[b][:],
                                mybir.ActivationFunctionType.Relu,
                                scale=dinv_sb[:, b : b + 1],
                            )
                            nc.tensor.matmul(
                                ps_p[:], u2_sb[:, b : b + 1], h1b[:],
                                start=(b == 0), stop=(b == NB - 1),
                                skip_group_check=True,
                            )

            # ---- p AllReduce
            p_sb = cpool.tile([1, F], f32)
            nc.vector.tensor_copy(p_sb[:], ps_p[:])
            nc.sync.dma_start(p_dram[:], p_sb[:])
            nc.gpsimd.collective_compute(
                "AllReduce", mybir.AluOpType.add,
                replica_groups=[groups_all],
                ins=[p_dram[:]], outs=[p_shared[:]],
            )
            p_row = cpool.tile([1, F], f32)
            nc.sync.dma_start(p_row[:], p_shared[:])
            id1 = cpool.tile([1, 1], f32)
            nc.vector.memset(id1[:], 1.0)
            psT = pspool.tile([F, 1], f32, tag="ps")
            nc.tensor.transpose(psT[:], p_row[:], id1[:])
            p_col = cpool.tile([F, 1], f32)
            nc.vector.tensor_copy(p_col[:], psT[:])

            # ---- replicated tail MLP
            tl = cpool
            def ld(e, shape, dt=f32):
                t = tl.tile(shape, dt, tag=f"c_{e.name}")
                nc.sync.dma_start(t[:], e[:])
                return t
            rates_sb = ld(rates_e, [16, 1]); encw1_sb = ld(encw1_e, [16, 8])
            encb1_sb = ld(encb1_e, [8, 1]); encw2_sb = ld(encw2_e, [8, F])
            encb2_sb = ld(encb2_e, [F, 1])
            w2a_sb = ld(w2a_e, [F, F]); w2b_sb = ld(w2b_e, [F, F])
            b2_sb = ld(b2_e, [F, 1]); s1_sb = ld(s1_e, [F, 1]); s2_sb = ld(s2_e, [F, 1])
            w3_sb = ld(w3_e, [F, F]); b3_sb = ld(b3_e, [F, 1])
            hidw_sb = ld(hidw_e, [F, 2 * F]); hidb_sb = ld(hidb_e, [F, 2])
            hid2wa_sb = ld(hid2wa_e, [F, F]); hid2wb_sb = ld(hid2wb_e, [F, F])
            hid2b_sb = ld(hid2b_e, [F, 1])
            finw_sb = ld(finw_e, [F, 2]); finb_sb = ld(finb_e, [2, 1])

            pst = pspool.tile([F, 2], f32, tag="ps")
            nc.tensor.matmul(pst[:8, 0:1], encw1_sb[:], rates_sb[:], start=True, stop=True)
            r1 = tl.tile([8, 1], f32)
            nc.scalar.activation(r1[:], pst[:8, 0:1],
                                 mybir.ActivationFunctionType.Relu, bias=encb1_sb[:])
            nc.tensor.matmul(pst[:, 1:2], encw2_sb[:], r1[:], start=True, stop=True)
            r2 = tl.tile([F, 1], f32)
            nc.vector.tensor_add(r2[:], pst[:, 1:2], encb2_sb[:])
            mr = tl.tile([F, 1], f32)
            nc.vector.tensor_mul(mr[:], r2[:], s2_sb[:])
            pst2 = pspool.tile([F, 1], f32, tag="ps")
            nc.tensor.matmul(pst2[:], w2a_sb[:], p_col[:], start=True, stop=False)
            nc.tensor.matmul(pst2[:], w2b_sb[:], mr[:], start=False, stop=True)
            sb2 = tl.tile([F, 1], f32)
            nc.vector.tensor_mul(sb2[:], b2_sb[:], s1_sb[:])
            qv = tl.tile([F, 1], f32)
            nc.vector.tensor_add(qv[:], pst2[:], sb2[:])
            nc.vector.tensor_scalar_mul(qv[:], qv[:], 1.0 / N)
            pst3 = pspool.tile([F, 1], f32, tag="ps")
            nc.tensor.matmul(pst3[:], w3_sb[:], qv[:], start=True, stop=True)
            m3 = tl.tile([F, 1], f32)
            nc.vector.tensor_add(m3[:], pst3[:], b3_sb[:])
            g1a = tl.tile([F, 1], f32); g1b = tl.tile([F, 1], f32)
            nc.tensor.matmul(pst[:, 0:1], hidw_sb[:, :F], m3[:], start=True, stop=True)
            nc.scalar.activation(g1a[:], pst[:, 0:1],
                                 mybir.ActivationFunctionType.Relu, bias=hidb_sb[:, 0:1])
            nc.tensor.matmul(pst[:, 1:2], hidw_sb[:, F:], m3[:], start=True, stop=True)
            nc.scalar.activation(g1b[:], pst[:, 1:2],
                                 mybir.ActivationFunctionType.Relu, bias=hidb_sb[:, 1:2])
            pst4 = pspool.tile([F, 1], f32, tag="ps")
            nc.tensor.matmul(pst4[:], hid2wa_sb[:], g1a[:], start=True, stop=False)
            nc.tensor.matmul(pst4[:], hid2wb_sb[:], g1b[:], start=False, stop=True)
            g2 = tl.tile([F, 1], f32)
            nc.scalar.activation(g2[:], pst4[:],
                                 mybir.ActivationFunctionType.Relu, bias=hid2b_sb[:])
            pst5 = pspool.tile([2, 1], f32, tag="ps")
            nc.tensor.matmul(pst5[:], finw_sb[:], g2[:], start=True, stop=True)
            outv = tl.tile([2, 1], f32)
            nc.vector.tensor_add(outv[:], pst5[:], finb_sb[:])
            nc.sync.dma_start(out_e[:], outv[:])

    nc.compile()
    return nc


_CACHE = {}
LAST_RESULTS = None


def kernel(**inputs):
    graph = np.asarray(inputs["graph"], np.float32)
    edge_index = np.asarray(inputs["edge_index"], np.int64)
    rates = np.asarray(inputs["rates"], np.float32)
    params = {k: np.asarray(v) for k, v in inputs.items()
              if k not in ("graph", "edge_index", "rates")}
    in_maps = _preprocess(graph, edge_index, rates, params)
    if "nc" not in _CACHE:
        _CACHE["nc"] = _build()
    nc = _CACHE["nc"]
    import os
    trace = bool(int(os.environ.get("GCN_TRACE", "0")))
    res = run_bass_kernel_spmd(nc, in_maps, list(range(N_CORES)), trace=trace)
    global LAST_RESULTS
    LAST_RESULTS = res
    out = np.asarray(res.results[0]["out"], np.float32).reshape(1, 2)
    return out


# revision 28
# speedup vs baseline: 4.2242x; 1.0160x over previous
"""GCN (3-layer + MLP head) on 8 Trainium2 NeuronCores.

Strategy (graph-parallel, dst-sharded, SWDGE-gather message passing):
  - Host renumbers nodes (LPT bin-packing on in-degree) so every 128-node
    dst block has total in-degree <= 2048; nodes shard 8 ways by new id.
  - Phase A (per core): Hs = dinv * (X_shard @ W1) as bf16 rows, kept in
    SBUF (self-loop term) and AllGathered -> full node table in HBM.
  - MP phase: each dst block's edges are sorted by src table row and split
    into 8 segments of exactly 256 edges (last = remainder); segment k of
    all blocks lies in a fixed ~32k-row window, so int16 gather indices
    cover it with zero per-cell padding.  Chunks of 16 blocks' k-segments
    (4096 idxs) feed dma_gather; per 128-edge group a one-hot S matrix
    (DVE is_equal vs iota) scatter-sums messages into the dst block via a
    PE matmul; segments accumulate into an SBUF f32 accum.
  - Self loops never enter the gather: an identity-matrix matmul adds the
    core's own Hs rows into each block's psum at segment 0.
  - GCN layers 2+3 feed only a global mean, so they collapse to weighted
    node sums (host-precomputed u2, S1, S2); p = sum_d u2[d]*relu(h1[d])
    is a PE reduction, AllReduced across cores; tiny MLP head replicated.
"""
import heapq
import numpy as np
import ml_dtypes

import concourse.bass as bass
import concourse.tile as tile
from concourse import bacc, mybir
from concourse.bass_utils import run_bass_kernel_spmd

N_CORES = 8
N = 100000
F = 128            # feature dim
BLK = 128          # dst-block size (psum partition dim)
NB = 98            # blocks per core
SHARD = NB * BLK   # 12544 rows per core (incl. pad rows)
ROWS = N_CORES * SHARD  # 100352 table rows
NSEG = 8           # segments per block
SEG = 256          # edges per segment (2 matmul groups)
GPB = NSEG * 2     # groups per block = 16
BPG = 14           # blocks per group (98 = 7*14)
CPP = 7            # block-groups
NCHUNK = NSEG * CPP
CH_FULL = BPG * SEG  # 3584 idxs
WIN = 32768
NQUEUES = 4
PAD_DST = 130.0

BF16 = ml_dtypes.bfloat16


def _win_base(k):
    return max(0, min(k * SHARD - 3584, ROWS - WIN))


# ----------------------------------------------------------------------------
# host preprocessing: renumber, normalize, sort, segment
# ----------------------------------------------------------------------------
def _preprocess(graph, edge_index, rates, params):
    src = np.asarray(edge_index[0], np.int64)
    dst = np.asarray(edge_index[1], np.int64)
    E = src.shape[0]

    # normalization scalars (f64, original ids; order-independent)
    deg = np.bincount(dst, minlength=N).astype(np.float64) + 1.0
    dinv = deg ** -0.5
    sq = deg ** 0.5
    u1 = dinv * (np.bincount(src, weights=dinv[dst], minlength=N) + dinv)
    y = u1 * dinv
    u2 = dinv * (np.bincount(src, weights=y[dst], minlength=N) + y)
    S1 = float(u1.sum())
    S2 = float(u2.sum())

    # LPT renumbering: balance per-block in-degree (target <= 2048)
    indeg = np.bincount(dst, minlength=N)
    nbins = N_CORES * NB
    order = np.argsort(-indeg, kind="stable")
    heap = [(0, 0, b) for b in range(nbins)]
    heapq.heapify(heap)
    binof = np.empty(N, np.int32)
    posof = np.empty(N, np.int32)
    for v in order:
        load, cnt, b = heapq.heappop(heap)
        binof[v] = b
        posof[v] = cnt
        cnt += 1
        load += int(indeg[v])
        if cnt < BLK:
            heapq.heappush(heap, (load, cnt, b))
    newid = binof.astype(np.int64) * BLK + posof
    # table row order is partition-major per core: row = c*SHARD + p*NB + b,
    # so the SBUF hs tile [128, NB, F] stores to DRAM as a single identity DMA
    c_ = newid // SHARD
    b_ = (newid % SHARD) // BLK
    p_ = newid % BLK
    trow = c_ * SHARD + p_ * NB + b_

    ns, nd = trow[src], newid[dst]
    cell = nd // BLK                      # global cell id = core*NB + block
    o = np.lexsort((ns, cell))
    cell_s, ns_s, off_s = cell[o], ns[o], (nd[o] % BLK).astype(np.float64)
    ccnt = np.bincount(cell_s, minlength=nbins)
    assert ccnt.max() <= NSEG * SEG, f"cell overflow {ccnt.max()}"
    cstart = np.zeros(nbins + 1, np.int64)
    cstart[1:] = np.cumsum(ccnt)

    # per-core edge tables in schedule order (block-group-major: ci = bg*8+k)
    idx16 = np.zeros((N_CORES, NCHUNK, 128, CH_FULL // 16), np.int16)
    dstid = np.full((N_CORES, NCHUNK, 128, CH_FULL // 128), PAD_DST, BF16)
    iv = np.zeros(CH_FULL, np.int64)
    ov = np.zeros(CH_FULL, np.float64)
    for c in range(N_CORES):
        for bg in range(CPP):
            blo, bhi = bg * BPG, (bg + 1) * BPG
            ncell = bhi - blo
            chn = ncell * SEG
            for k in range(NSEG):
                base = _win_base(k)
                iv[:chn] = 0
                ov[:chn] = PAD_DST
                for ci2 in range(ncell):
                    g_cell = c * NB + blo + ci2
                    s0, T = cstart[g_cell], ccnt[g_cell]
                    a, b2 = k * SEG, min((k + 1) * SEG, T)
                    if a >= b2:
                        continue
                    take = b2 - a
                    rel = ns_s[s0 + a : s0 + b2] - base
                    assert rel.min() >= 0 and rel.max() < WIN, (c, k, bg, ci2)
                    p0 = ci2 * SEG
                    iv[p0 : p0 + take] = rel
                    ov[p0 : p0 + take] = off_s[s0 + a : s0 + b2]
                ci = bg * NSEG + k
                e = np.arange(chn)
                tmp = np.zeros((16, CH_FULL // 16), np.int16)
                tmp[e % 16, e // 16] = iv[:chn]
                idx16[c, ci] = np.tile(tmp, (8, 1))
                dstid[c, ci][e % 128, e // 128] = ov[:chn].astype(BF16)

    # phase A inputs (new ordering, padded)
    X = np.asarray(graph, np.float32)
    inv = np.full(ROWS, -1, np.int64)
    inv[newid] = np.arange(N)
    xt = np.zeros((N_CORES, F, SHARD), np.float32)
    dinv_pm = np.zeros((N_CORES, BLK, NB), np.float32)
    u2_pm = np.zeros((N_CORES, BLK, NB), np.float32)
    sqdeg = np.zeros((N_CORES, 1, SHARD), np.float32)
    dv = np.zeros(ROWS)
    uv = np.zeros(ROWS)
    sv = np.zeros(ROWS)
    real = inv >= 0
    dv[real] = dinv[inv[real]]
    uv[real] = u2[inv[real]]
    sv[real] = sq[inv[real]]
    for c in range(N_CORES):
        rows = inv[c * SHARD : (c + 1) * SHARD]
        m = rows >= 0
        xt[c][:, m] = X[rows[m]].T
        dinv_pm[c] = dv[c * SHARD : (c + 1) * SHARD].reshape(NB, BLK).T
        u2_pm[c] = uv[c * SHARD : (c + 1) * SHARD].reshape(NB, BLK).T
        sqdeg[c, 0] = sv[c * SHARD : (c + 1) * SHARD]

    p = params
    col = lambda v: np.asarray(v, np.float32).reshape(-1, 1)
    iota = np.tile(np.arange(BLK, dtype=BF16)[None, :], (128, 1))
    ident = np.eye(128, dtype=BF16)
    common = dict(
        w1=np.asarray(p["conv1_W"], np.float32),
        b1row=np.asarray(p["conv1_b"], BF16).reshape(1, F),
        iota=iota,
        ident=ident,
        rates_col=col(rates),
        encw1=np.asarray(p["enc_W1"], np.float32),
        encb1=col(p["enc_b1"]),
        encw2=np.asarray(p["enc_W2"], np.float32),
        encb2=col(p["enc_b2"]),
        w2a=np.asarray(p["conv2_W"], np.float32)[:F],
        w2b=np.asarray(p["conv2_W"], np.float32)[F:],
        b2col=col(p["conv2_b"]),
        s1col=np.full((F, 1), S1, np.float32),
        s2col=np.full((F, 1), S2, np.float32),
        w3=np.asarray(p["conv3_W"], np.float32),
        b3col=col(p["conv3_b"]),
        hidw=np.asarray(p["hid_W"], np.float32),
        hidb=np.asarray(p["hid_b"], np.float32).reshape(2, F).T,
        hid2wa=np.asarray(p["hid2_W"], np.float32)[:F],
        hid2wb=np.asarray(p["hid2_W"], np.float32)[F:],
        hid2b=col(p["hid2_b"]),
        finw=np.asarray(p["fin_W"], np.float32),
        finb=col(p["fin_b"]),
    )
    in_maps = []
    for c in range(N_CORES):
        m = dict(common)
        m.update(
            xt=xt[c], sqdeg=sqdeg[c].astype(BF16), dinv=dinv_pm[c],
            u2c=u2_pm[c], idx16=idx16[c],
            dstid=np.ascontiguousarray(dstid[c].transpose(1, 0, 2)),
        )
        in_maps.append(m)
    return in_maps


# ----------------------------------------------------------------------------
# device program
# ----------------------------------------------------------------------------
def _build():
    f32, bf16, i16 = mybir.dt.float32, mybir.dt.bfloat16, mybir.dt.int16

    nc = bacc.Bacc("TRN2", target_bir_lowering=False, debug=False,
                   num_devices=N_CORES, num_swdge_queues=NQUEUES)
    I = lambda name, shape, dt=f32: nc.dram_tensor(name, shape, dt, kind="ExternalInput")
    xt_e = I("xt", [F, SHARD])
    w1_e = I("w1", [F, F]); b1_e = I("b1row", [1, F], bf16)
    sq_e = I("sqdeg", [1, SHARD], bf16)
    dinv_e = I("dinv", [BLK, NB]); u2_e = I("u2c", [BLK, NB])
    idx_e = I("idx16", [NCHUNK, 128, CH_FULL // 16], i16)
    dst_e = I("dstid", [128, NCHUNK, CH_FULL // 128], bf16)
    iota_e = I("iota", [128, BLK], bf16)
    ident_e = I("ident", [128, 128], bf16)
    rates_e = I("rates_col", [16, 1])
    encw1_e = I("encw1", [16, 8]); encb1_e = I("encb1", [8, 1])
    encw2_e = I("encw2", [8, F]); encb2_e = I("encb2", [F, 1])
    w2a_e = I("w2a", [F, F]); w2b_e = I("w2b", [F, F]); b2_e = I("b2col", [F, 1])
    s1_e = I("s1col", [F, 1]); s2_e = I("s2col", [F, 1])
    w3_e = I("w3", [F, F]); b3_e = I("b3col", [F, 1])
    hidw_e = I("hidw", [F, 2 * F]); hidb_e = I("hidb", [F, 2])
    hid2wa_e = I("hid2wa", [F, F]); hid2wb_e = I("hid2wb", [F, F])
    hid2b_e = I("hid2b", [F, 1])
    finw_e = I("finw", [F, 2]); finb_e = I("finb", [2, 1])
    out_e = nc.dram_tensor("out", [2, 1], f32, kind="ExternalOutput")

    hs_shard = nc.dram_tensor("hs_shard", [128, NB * F], bf16)
    hs_full = nc.dram_tensor("hs_full", [ROWS, F], bf16, addr_space="Shared")
    p_dram = nc.dram_tensor("p_dram", [1, F], f32)
    p_shared = nc.dram_tensor("p_shared", [1, F], f32, addr_space="Shared")
    groups_all = list(range(N_CORES))

    with tile.TileContext(nc) as tc:
        with (
            tc.tile_pool(name="const", bufs=1) as cpool,
            tc.tile_pool(name="xt", bufs=3) as xtpool,
            tc.tile_pool(name="hself", bufs=1) as hpool,
            tc.tile_pool(name="work", bufs=8) as wpool,
            tc.tile_pool(name="gat", bufs=8) as gpool,
            tc.tile_pool(name="sstile", bufs=4) as spool,
            tc.tile_pool(name="ps", bufs=2, space="PSUM") as pspool,
            tc.tile_pool(name="cellps", bufs=1, space="PSUM") as cpspool,
            tc.tile_pool(name="psp", bufs=1, space="PSUM") as psppool,
        ):
            # ---- constants
            w1_sb = cpool.tile([F, F], f32); nc.sync.dma_start(w1_sb[:], w1_e[:])
            b1_sb = cpool.tile([1, F], bf16); nc.sync.dma_start(b1_sb[:], b1_e[:])
            sq_sb = cpool.tile([1, SHARD], bf16); nc.sync.dma_start(sq_sb[:], sq_e[:])
            dinv_sb = cpool.tile([BLK, NB], f32); nc.sync.dma_start(dinv_sb[:], dinv_e[:])
            u2_sb = cpool.tile([BLK, NB], f32); nc.sync.dma_start(u2_sb[:], u2_e[:])
            iota_sb = cpool.tile([128, BLK], bf16); nc.sync.dma_start(iota_sb[:], iota_e[:])
            ident_sb = cpool.tile([128, 128], bf16); nc.sync.dma_start(ident_sb[:], ident_e[:])
            # all chunks' dst-offset columns, preloaded in one DMA
            dsta_sb = cpool.tile([128, NCHUNK * (CH_FULL // 128)], bf16)
            nc.sync.dma_start(dsta_sb[:], dst_e[:])

            # ---- phase A: Hs = dinv * (X @ W1), bf16, kept in SBUF
            hs_self = hpool.tile([128, NB, F], bf16)
            for i in range(14):
                xts = xtpool.tile([F, 7 * BLK], f32, tag="xts")
                nc.sync.dma_start(xts[:], xt_e[:, i * 7 * BLK : (i + 1) * 7 * BLK])
                for j in range(7):
                    b = i * 7 + j
                    psA = pspool.tile([BLK, F], f32, tag="ps")
                    nc.tensor.matmul(psA[:], xts[:, j * BLK : (j + 1) * BLK],
                                     w1_sb[:], start=True, stop=True)
                    nc.scalar.activation(
                        hs_self[:, b, :], psA[:], mybir.ActivationFunctionType.Copy,
                        scale=dinv_sb[:, b : b + 1],
                    )
                if i % 2 == 1:
                    # store finished 14-block stretch while phase A continues
                    lo = (i - 1) * 7 * F
                    nc.sync.dma_start(hs_shard[:, lo : lo + 14 * F],
                                      hs_self[:, (i - 1) * 7 : (i + 1) * 7, :])

            # ---- AllGather the node table
            nc.gpsimd.collective_compute(
                "AllGather", mybir.AluOpType.bypass,
                replica_groups=[groups_all],
                ins=[hs_shard[:]], outs=[hs_full[:]],
            )

            # ---- message passing: 7 block-groups x 8 segment chunks
            # each block's 16 matmul groups accumulate purely in PSUM
            ps_p = psppool.tile([1, F], f32)
            for bg in range(CPP):
                blo = bg * BPG
                ncell = BPG
                chn = ncell * SEG
                psbs = {}
                for t in range((ncell + 3) // 4):
                    pst_new = cpspool.tile([BLK, 4 * F], f32, tag=f"cps{t}")
                    for q in range(min(4, ncell - t * 4)):
                        psbs[blo + t * 4 + q] = pst_new[:, q * F : (q + 1) * F]
                for k in range(NSEG):
                    ci = bg * NSEG + k
                    base = _win_base(k)
                    idxt = wpool.tile([128, CH_FULL // 16], i16, tag="idxt")
                    nc.sync.dma_start(idxt[:, : chn // 16], idx_e[ci, :, : chn // 16])
                    G = gpool.tile([128, CH_FULL // 128, F], bf16, tag="G")
                    nc.gpsimd.dma_gather(
                        out_ap=G[:, : chn // 128, :],
                        in_ap=hs_full[base : base + WIN, :],
                        idxs_ap=idxt[:, : chn // 16],
                        num_idxs=chn, num_idxs_reg=chn, elem_size=F,
                        single_packet=False, queue_num=(ci % NQUEUES),
                    )
                    Sts = []
                    for sg in range((chn // 128 + 3) // 4):
                        S4 = spool.tile([128, 4, BLK], bf16, tag=f"S{sg % 2}")
                        iap = iota_sb[:]
                        in0 = bass.AP(iap.tensor, iap.offset,
                                      [iap.ap[0], [0, 4], iap.ap[1]])
                        c0 = ci * (CH_FULL // 128) + sg * 4
                        dap = dsta_sb[:, c0 : c0 + 4]
                        in1 = bass.AP(dap.tensor, dap.offset,
                                      [dap.ap[0], dap.ap[1], [0, 128]])
                        nc.vector.tensor_tensor(S4[:], in0, in1,
                                                mybir.AluOpType.is_equal)
                        Sts.append(S4)
                    for ci2 in range(ncell):
                        b = blo + ci2
                        psb = psbs[b]
                        if k == 0:
                            # bias (sqrt(deg) (x) b1) + self-loop rows.
                            # start=True zeroes the WHOLE psum bank, so only
                            # the first slice of each 4-block bank sets it.
                            nc.tensor.matmul(
                                psb[:], sq_sb[:, b * BLK : (b + 1) * BLK],
                                b1_sb[:], start=(ci2 % 4 == 0), stop=False,
                                skip_group_check=True,
                            )
                            nc.tensor.matmul(
                                psb[:], ident_sb[:], hs_self[:, b, :],
                                start=False, stop=False, skip_group_check=True,
                            )
                        for g2 in range(2):
                            g = ci2 * 2 + g2
                            nc.tensor.matmul(
                                psb[:], Sts[g // 4][:, g % 4, :], G[:, g, :],
                                start=False, stop=(k == NSEG - 1 and g2 == 1),
                                skip_group_check=True,
                            )
                    if k == NSEG - 1:
                        for ci2 in range(ncell):
                            b = blo + ci2
                            h1b = spool.tile([BLK, F], f32, tag="h1b")
                            nc.scalar.activation(
                                h1b[:], psbs[b][:],
                                mybir.ActivationFunctionType.Relu,
                                scale=dinv_sb[:, b : b + 1],
                            )
                            nc.tensor.matmul(
                                ps_p[:], u2_sb[:, b : b + 1], h1b[:],
                                start=(b == 0), stop=(b == NB - 1),
                                skip_group_check=True,
                            )

            # ---- p AllReduce
            p_sb = cpool.tile([1, F], f32)
            nc.vector.tensor_copy(p_sb[:], ps_p[:])
            nc.sync.dma_start(p_dram[:], p_sb[:])
            nc.gpsimd.collective_compute(
                "AllReduce", mybir.AluOpType.add,
                replica_groups=[groups_all],
                ins=[p_dram[:]], outs=[p_shared[:]],
            )
            p_row = cpool.tile([1, F], f32)
            nc.sync.dma_start(p_row[:], p_shared[:])
            id1 = cpool.tile([1, 1], f32)
            nc.vector.memset(id1[:], 1.0)
            psT = pspool.tile([F, 1], f32, tag="ps")
            nc.tensor.transpose(psT[:], p_row[:], id1[:])
            p_col = cpool.tile([F, 1], f32)
            nc.vector.tensor_copy(p_col[:], psT[:])

            # ---- replicated tail MLP
            tl = cpool
            def ld(e, shape, dt=f32):
                t = tl.tile(shape, dt, tag=f"c_{e.name}")
                nc.sync.dma_start(t[:], e[:])
                return t
            rates_sb = ld(rates_e, [16, 1]); encw1_sb = ld(encw1_e, [16, 8])
            encb1_sb = ld(encb1_e, [8, 1]); encw2_sb = ld(encw2_e, [8, F])
            encb2_sb = ld(encb2_e, [F, 1])
            w2a_sb = ld(w2a_e, [F, F]); w2b_sb = ld(w2b_e, [F, F])
            b2_sb = ld(b2_e, [F, 1]); s1_sb = ld(s1_e, [F, 1]); s2_sb = ld(s2_e, [F, 1])
            w3_sb = ld(w3_e, [F, F]); b3_sb = ld(b3_e, [F, 1])
            hidw_sb = ld(hidw_e, [F, 2 * F]); hidb_sb = ld(hidb_e, [F, 2])
            hid2wa_sb = ld(hid2wa_e, [F, F]); hid2wb_sb = ld(hid2wb_e, [F, F])
            hid2b_sb = ld(hid2b_e, [F, 1])
            finw_sb = ld(finw_e, [F, 2]); finb_sb = ld(finb_e, [2, 1])

            pst = pspool.tile([F, 2], f32, tag="ps")
            nc.tensor.matmul(pst[:8, 0:1], encw1_sb[:], rates_sb[:], start=True, stop=True)
            r1 = tl.tile([8, 1], f32)
            nc.scalar.activation(r1[:], pst[:8, 0:1],
                                 mybir.ActivationFunctionType.Relu, bias=encb1_sb[:])
            nc.tensor.matmul(pst[:, 1:2], encw2_sb[:], r1[:], start=True, stop=True)
            r2 = tl.tile([F, 1], f32)
            nc.vector.tensor_add(r2[:], pst[:, 1:2], encb2_sb[:])
            mr = tl.tile([F, 1], f32)
            nc.vector.tensor_mul(mr[:], r2[:], s2_sb[:])
            pst2 = pspool.tile([F, 1], f32, tag="ps")
            nc.tensor.matmul(pst2[:], w2a_sb[:], p_col[:], start=True, stop=False)
            nc.tensor.matmul(pst2[:], w2b_sb[:], mr[:], start=False, stop=True)
            sb2 = tl.tile([F, 1], f32)
            nc.vector.tensor_mul(sb2[:], b2_sb[:], s1_sb[:])
            qv = tl.tile([F, 1], f32)
            nc.vector.tensor_add(qv[:], pst2[:], sb2[:])
            nc.vector.tensor_scalar_mul(qv[:], qv[:], 1.0 / N)
            pst3 = pspool.tile([F, 1], f32, tag="ps")
            nc.tensor.matmul(pst3[:], w3_sb[:], qv[:], start=True, stop=True)
            m3 = tl.tile([F, 1], f32)
            nc.vector.tensor_add(m3[:], pst3[:], b3_sb[:])
            g1a = tl.tile([F, 1], f32); g1b = tl.tile([F, 1], f32)
            nc.tensor.matmul(pst[:, 0:1], hidw_sb[:, :F], m3[:], start=True, stop=True)
            nc.scalar.activation(g1a[:], pst[:, 0:1],
                                 mybir.ActivationFunctionType.Relu, bias=hidb_sb[:, 0:1])
            nc.tensor.matmul(pst[:, 1:2], hidw_sb[:, F:], m3[:], start=True, stop=True)
            nc.scalar.activation(g1b[:], pst[:, 1:2],
                                 mybir.ActivationFunctionType.Relu, bias=hidb_sb[:, 1:2])
            pst4 = pspool.tile([F, 1], f32, tag="ps")
            nc.tensor.matmul(pst4[:], hid2wa_sb[:], g1a[:], start=True, stop=False)
            nc.tensor.matmul(pst4[:], hid2wb_sb[:], g1b[:], start=False, stop=True)
            g2 = tl.tile([F, 1], f32)
            nc.scalar.activation(g2[:], pst4[:],
                                 mybir.ActivationFunctionType.Relu, bias=hid2b_sb[:])
            pst5 = pspool.tile([2, 1], f32, tag="ps")
            nc.tensor.matmul(pst5[:], finw_sb[:], g2[:], start=True, stop=True)
            outv = tl.tile([2, 1], f32)
            nc.vector.tensor_add(outv[:], pst5[:], finb_sb[:])
            nc.sync.dma_start(out_e[:], outv[:])

    nc.compile()
    return nc


_CACHE = {}
LAST_RESULTS = None


def kernel(**inputs):
    graph = np.asarray(inputs["graph"], np.float32)
    edge_index = np.asarray(inputs["edge_index"], np.int64)
    rates = np.asarray(inputs["rates"], np.float32)
    params = {k: np.asarray(v) for k, v in inputs.items()
              if k not in ("graph", "edge_index", "rates")}
    in_maps = _preprocess(graph, edge_index, rates, params)
    if "nc" not in _CACHE:
        _CACHE["nc"] = _build()
    nc = _CACHE["nc"]
    import os
    trace = bool(int(os.environ.get("GCN_TRACE", "0")))
    res = run_bass_kernel_spmd(nc, in_maps, list(range(N_CORES)), trace=trace)
    global LAST_RESULTS
    LAST_RESULTS = res
    out = np.asarray(res.results[0]["out"], np.float32).reshape(1, 2)
    return out
